# revision 1
# baseline (speedup 1.0000x reference)
# Multi-head attention (RoPE, causal) Trainium2 Bass kernel.
# B=2, S=2048, D=1024, 16 heads, hd=64, fp32 I/O.
#
# Sharding: 32 (batch, head) units over 8 cores -> each core gets one batch
# and 4 heads. Each core computes its 4 heads' attention output and the
# partial out-projection (sum over its heads); the host sums the 4 partials
# per batch and adds the bias constant.
#
# Self-contained: all shapes/sharding hardcoded; no sibling imports.

import numpy as np

import concourse.bass as bass  # noqa: F401
import concourse.mybir as mybir
import concourse.tile as tile
from concourse import bacc, bass_utils

F32 = mybir.dt.float32
BF16 = mybir.dt.bfloat16
EXP = mybir.ActivationFunctionType.Exp

B = 2
S = 2048
D = 1024
NHEADS = 16
HD = 64
HPC = 4  # heads per core
NCORES = 8
NPAIR = 2  # head pairs per core
P = 128
CH = 512  # q chunk
THETA = 10000.0
QKVW = 3 * HPC * HD  # 768

# module-level knobs for test harness
TRACE = False
LAST_RESULTS = None

_PROGRAM_CACHE = {}


def build_program(s=S, mm_fast=True):
    """Build + compile the single-core SPMD program.

    mm_fast=True: bf16 for all PE operands (fp32 PSUM accumulation).
    mm_fast=False: everything fp32 (4x slower matmuls, reference-grade).
    """
    nt = s // P      # s-tiles
    nch = s // CH    # q chunks
    kt = D // P      # 8 contraction tiles
    PD = BF16 if mm_fast else F32

    nc = bacc.Bacc(
        "TRN2", target_bir_lowering=False, debug=False, enable_asserts=False
    )

    # ---- DRAM I/O ----
    xt_d = nc.dram_tensor("xt", [P, kt * s], PD, kind="ExternalInput").ap()
    wt_d = nc.dram_tensor("wt", [P, kt * QKVW], PD, kind="ExternalInput").ap()
    biasqk_d = nc.dram_tensor("biasqk", [P, 512], PD, kind="ExternalInput").ap()
    ropec_d = nc.dram_tensor("ropec", [P, nt * 256], PD, kind="ExternalInput").ap()
    ropes_d = nc.dram_tensor("ropes", [P, nt * 256], PD, kind="ExternalInput").ap()
    trimask_d = nc.dram_tensor("trimask", [P, P], PD, kind="ExternalInput").ap()
    ident_d = nc.dram_tensor("ident", [P, P], PD, kind="ExternalInput").ap()
    wo_d = nc.dram_tensor("wo", [P, NPAIR * D], PD, kind="ExternalInput").ap()
    out_d = nc.dram_tensor("outp", [s, D], F32, kind="ExternalOutput").ap()

    from contextlib import ExitStack

    with tile.TileContext(nc) as tc, ExitStack() as ctx:
        const = ctx.enter_context(tc.tile_pool(name="const", bufs=1))

        # const loads ride the ACT HWDGE ring so they don't serialize with
        # the x-tile loads on the sync ring
        wt_sb = const.tile([P, kt * QKVW], PD)
        qtr = kt * QKVW // 4
        for qi in range(4):
            nc.scalar.dma_start(
                wt_sb[:, qi * qtr : (qi + 1) * qtr],
                wt_d[:, qi * qtr : (qi + 1) * qtr],
            )
        biasqk_sb = const.tile([P, 512], PD)
        nc.scalar.dma_start(biasqk_sb[:], biasqk_d[:])
        ident_sb = const.tile([P, P], PD)
        nc.scalar.dma_start(ident_sb[:], ident_d[:])
        ropec_sb = const.tile([P, nt * 256], PD)
        nc.scalar.dma_start(ropec_sb[:], ropec_d[:])
        ropes_sb = const.tile([P, nt * 256], PD)
        nc.scalar.dma_start(ropes_sb[:], ropes_d[:])
        trimask_sb = const.tile([P, P], PD)
        nc.scalar.dma_start(trimask_sb[:], trimask_d[:])
        wo_sb = const.tile([P, NPAIR * D], PD)
        nc.scalar.dma_start(wo_sb[:], wo_d[:])

        # persistent activations
        # qkt2 blocks: [Qpack0, Qpack1, Kpack0, Kpack1] each [128 (2 heads*hd), s]
        qkt2 = const.tile([P, 4 * s], PD)
        # vone: per s-tile [128, 4*65]; per head 64 V cols + ones col
        vone = const.tile([P, nt * (HPC * 65)], PD)
        # otn2: O^T packs (normalized in place): [128 (2 heads*hd), s] per pair
        otn2 = const.tile([P, NPAIR * s], PD)

        # ones columns of vone
        vone_v = vone.rearrange("p (t h c) -> p t h c", t=nt, h=HPC)
        nc.gpsimd.memset(vone_v[:, :, :, 64], 1.0)

        # half-selector columns for the rank-1 denominator broadcast:
        # halfsel[0, 0:128] selects partitions 0-63, [128:256] selects 64-127
        halfsel = const.tile([1, 256], PD)
        nc.gpsimd.memset(halfsel[:, 0:64], 1.0)
        nc.gpsimd.memset(halfsel[:, 64:192], 0.0)
        nc.gpsimd.memset(halfsel[:, 192:256], 1.0)


        # ---------------- Phase A: QKV + RoPE + transposes ----------------
        bpsum = ctx.enter_context(tc.tile_pool(name="bpsum", bufs=2, space="PSUM"))
        awork = ctx.enter_context(tc.tile_pool(name="awork", bufs=3))

        def emit_A(st):
            if True:
                xts = awork.tile([P, D], PD, name="xts")
                nc.sync.dma_start(xts[:], xt_d[:, st * D : (st + 1) * D])
                psA = bpsum.tile([P, 1024], F32, name="psA", tag="big")
                for k in range(kt):
                    lhs = xts[:, k * P : (k + 1) * P]
                    rhs = wt_sb[:, k * QKVW : k * QKVW + QKVW]
                    nc.tensor.matmul(
                        psA[:, 0:512],
                        lhsT=lhs,
                        rhs=rhs[:, 0:512],
                        start=(k == 0),
                        stop=(k == kt - 1),
                    )
                    nc.tensor.matmul(
                        psA[:, 512:768],
                        lhsT=lhs,
                        rhs=rhs[:, 512:768],
                        start=(k == 0),
                        stop=(k == kt - 1),
                    )
                # evict V into vone slots (no V bias: folded into host const)
                nc.scalar.copy(
                    vone_v[:, st, :, 0:64],
                    psA[:, 512:768].rearrange("p (h c) -> p h c", h=HPC),
                )
                # evict Q,K with bias
                qk = awork.tile([P, 512], PD, name="qk")
                nc.vector.tensor_add(qk[:], psA[:, 0:512], biasqk_sb[:])
                # rope: rot = qk*cos + swap(qk)*sin
                sw = awork.tile([P, 512], PD, name="sw")
                qk_v = qk.rearrange("p (n two) -> p n two", two=2)
                sw_v = sw.rearrange("p (n two) -> p n two", two=2)
                nc.vector.tensor_copy(sw_v[:, :, 0], qk_v[:, :, 1])
                nc.vector.tensor_copy(sw_v[:, :, 1], qk_v[:, :, 0])
                rc = ropec_sb[:, st * 256 : (st + 1) * 256]
                rs = ropes_sb[:, st * 256 : (st + 1) * 256]
                rot = awork.tile([P, 512], PD, name="rot")
                nc.vector.tensor_mul(rot[:, 0:256], qk[:, 0:256], rc)
                nc.vector.tensor_mul(rot[:, 256:512], qk[:, 256:512], rc)
                nc.gpsimd.tensor_mul(sw[:, 0:256], sw[:, 0:256], rs)
                nc.gpsimd.tensor_mul(sw[:, 256:512], sw[:, 256:512], rs)
                nc.vector.tensor_add(rot[:], rot[:], sw[:])
                # transpose the 4 pack blocks -> qkt2
                tp = bpsum.tile([P, 512], PD, name="tp", tag="acc")
                for b in range(4):
                    nc.tensor.transpose(
                        tp[:, b * P : (b + 1) * P],
                        rot[:, b * P : (b + 1) * P],
                        ident_sb[:],
                    )
                nc.scalar.copy(
                    qkt2.rearrange("p (b s) -> p b s", b=4)[
                        :, :, st * P : (st + 1) * P
                    ],
                    tp.rearrange("p (b j) -> p b j", b=4),
                )

        # ---------------- Phase B: attention ----------------
        bwork = ctx.enter_context(tc.tile_pool(name="bwork", bufs=3))
        fxwork = ctx.enter_context(tc.tile_pool(name="fxwork", bufs=2))
        # softmax denominator collection: per (pair, J): [headA 512 | headB 512]
        dall = const.tile([1, NPAIR * nch * 1024], F32)
        dallinv = const.tile([1, NPAIR * nch * 1024], PD)

        def emit_BJ(p, j):
            q_pack = qkt2[:, p * s : (p + 1) * s]
            k_pack = qkt2[:, (2 + p) * s : (3 + p) * s]
            ot2 = bpsum.tile([P, 1024], F32, name="ot2", tag="acc")
            mlast = 4 * j + 3
            for m in range(4 * j + 4):
                sc2 = bpsum.tile([P, 1024], F32, name="sc2", tag="big")
                kA = k_pack[0:64, m * P : (m + 1) * P]
                kB = k_pack[64:128, m * P : (m + 1) * P]
                qA = q_pack[0:64, j * CH : (j + 1) * CH]
                qB = q_pack[64:128, j * CH : (j + 1) * CH]
                nc.tensor.matmul(sc2[:, 0:512], lhsT=kA, rhs=qA)
                nc.tensor.matmul(sc2[:, 512:1024], lhsT=kB, rhs=qB)
                at2 = bwork.tile([P, 1024], PD, name="at2")
                # columns below the diagonal block are dead: skip them in
                # exp and in the AV accumulation entirely
                off = m * P - j * CH if m >= 4 * j else 0
                if off > 0:
                    sc_v = sc2.rearrange("p (h q) -> p h q", h=2)
                    at_v = at2.rearrange("p (h q) -> p h q", h=2)
                    nc.scalar.activation(
                        at_v[:, :, off:512],
                        sc_v[:, :, off:512],
                        EXP,
                        scale=0.125,
                    )
                else:
                    nc.scalar.activation(at2[:], sc2[:], EXP, scale=0.125)
                if m >= 4 * j:
                    nc.vector.tensor_mul(
                        at2[:, off : off + P],
                        at2[:, off : off + P],
                        trimask_sb[:],
                    )
                    nc.vector.tensor_mul(
                        at2[:, 512 + off : 512 + off + P],
                        at2[:, 512 + off : 512 + off + P],
                        trimask_sb[:],
                    )
                vA = vone_v[:, m, 2 * p, :]
                vB = vone_v[:, m, 2 * p + 1, :]
                nc.tensor.matmul(
                    ot2[0:65, off:512],
                    lhsT=vA,
                    rhs=at2[:, off:512],
                    start=(m == 0),
                    stop=(m == mlast),
                )
                nc.tensor.matmul(
                    ot2[0:65, 512 + off : 1024],
                    lhsT=vB,
                    rhs=at2[:, 512 + off : 1024],
                    start=(m == 0),
                    stop=(m == mlast),
                )
            # ---- fixup: evict OT halves + denominators (per J) ----
            nc.vector.tensor_copy(
                otn2[0:64, p * s + j * CH : p * s + (j + 1) * CH],
                ot2[0:64, 0:512],
            )
            stgB = fxwork.tile([64, 512], PD, name="stgB")
            nc.vector.tensor_copy(stgB[:], ot2[0:64, 512:1024])
            nc.sync.dma_start(
                otn2[64:128, p * s + j * CH : p * s + (j + 1) * CH],
                stgB[:],
            )
            dslot = (p * nch + j) * 1024
            nc.vector.tensor_copy(
                dall[0:1, dslot : dslot + 512], ot2[64:65, 0:512]
            )
            nc.vector.tensor_copy(
                dall[0:1, dslot + 512 : dslot + 1024], ot2[64:65, 512:1024]
            )
            # reciprocal in partition-parallel layout: scatter the 1024 D
            # values across partitions, one wide reciprocal, scatter back
            dPj = fxwork.tile([P, 8], F32, name="dPj")
            nc.sync.dma_start(
                dPj[:],
                dall[0:1, dslot : dslot + 1024].rearrange("o (a b) -> o a b", a=P),
            )
            dPq = fxwork.tile([P, 8], F32, name="dPq")
            nc.vector.reciprocal(dPq[:], dPj[:])
            dPc = fxwork.tile([P, 8], PD, name="dPc")
            with nc.allow_low_precision("softmax denominators"):
                nc.vector.tensor_copy(dPc[:], dPq[:])
            nc.sync.dma_start(
                dallinv[0:1, dslot : dslot + 1024].rearrange(
                    "o (a b) -> o a b", a=P
                ),
                dPc[:],
            )
        def emit_final(p, j):
            dslot = (p * nch + j) * 1024
            # broadcast denominators across partitions via rank-1 matmuls
            # (rows 0-63 get head A's dinv, 64-127 head B's) and normalize
            dvb = bpsum.tile([P, 512], F32, name="dvb", tag="acc")
            nc.tensor.matmul(
                dvb[:],
                lhsT=halfsel[0:1, 0:128],
                rhs=dallinv[0:1, dslot : dslot + 512],
                start=True,
                stop=False,
            )
            nc.tensor.matmul(
                dvb[:],
                lhsT=halfsel[0:1, 128:256],
                rhs=dallinv[0:1, dslot + 512 : dslot + 1024],
                start=False,
                stop=True,
            )
            nc.vector.tensor_mul(
                otn2[:, p * s + j * CH : p * s + (j + 1) * CH],
                otn2[:, p * s + j * CH : p * s + (j + 1) * CH],
                dvb[:],
            )


        cwork = ctx.enter_context(tc.tile_pool(name="cwork", bufs=3))

        def emit_C(g):
            # out projection for q-tiles 4g..4g+3 (needs both pairs' chunk-g
            # finals done)
            for qt in range(4 * g, min(4 * g + 4, nt)):
                outsb = cwork.tile([P, D], F32, name="outsb")
                for dc in range(2):
                    pr = bpsum.tile([P, 512], F32, name="pr", tag="big")
                    for p in range(NPAIR):
                        nc.tensor.matmul(
                            pr[:],
                            lhsT=otn2[:, p * s + qt * P : p * s + (qt + 1) * P],
                            rhs=wo_sb[:, p * D + dc * 512 : p * D + (dc + 1) * 512],
                            start=(p == 0),
                            stop=(p == NPAIR - 1),
                        )
                    if dc == 0:
                        nc.vector.tensor_copy(outsb[:, 0:512], pr[:])
                    else:
                        nc.scalar.copy(outsb[:, 512:1024], pr[:])
                nc.sync.dma_start(out_d[qt * P : (qt + 1) * P, :], outsb[:])

        # sequential A then B; fixup finals pipeline one chunk behind so the
        # rank-1 broadcast + normalize never gate the next chunk's PSUM; the
        # out-projection interleaves per chunk as soon as both pairs' finals
        # for that q-range are emitted
        for st in range(nt):
            emit_A(st)
        pending = None
        for p in range(NPAIR):
            for j in range(nch):
                emit_BJ(p, j)
                if pending is not None:
                    emit_final(*pending)
                    if pending[0] == 1:
                        emit_C(pending[1])
                pending = (p, j)
        emit_final(*pending)
        emit_C(pending[1])

    nc.compile()
    return nc


def get_program(s=S, mm_fast=True):
    key = (s, mm_fast)
    if key not in _PROGRAM_CACHE:
        _PROGRAM_CACHE[key] = build_program(s, mm_fast)
    return _PROGRAM_CACHE[key]


def _to_pd(a, mm_fast):
    if mm_fast:
        import ml_dtypes

        return np.ascontiguousarray(a).astype(ml_dtypes.bfloat16)
    return np.ascontiguousarray(a).astype(np.float32)


def prep_core_inputs(x, w_qkv, b_qkv, w_out, core, s=S, mm_fast=True):
    """Build the per-core input map (numpy, host-side sharding/layout)."""
    nt = s // P
    kt = D // P
    b = core // 4
    heads = [(core % 4) * HPC + i for i in range(HPC)]

    xb = np.ascontiguousarray(x[b][:s])  # [s, D]
    # xt[p, st*D + k*128 + j] = x[st*128+j, k*128+p]  (contiguous per s-tile)
    xt = np.ascontiguousarray(
        xb.reshape(nt, P, kt, P).transpose(3, 0, 2, 1).reshape(P, nt * kt * P)
    )

    rows = []
    for part in range(3):
        for h in heads:
            rows.extend(range(part * D + h * HD, part * D + (h + 1) * HD))
    w_sel = w_qkv[rows]  # [768, 1024]
    b_sel = b_qkv[rows]  # [768]
    # wt[p, k*768 + n] = w_sel[n, k*128+p]
    wt = np.ascontiguousarray(
        w_sel.T.reshape(kt, P, QKVW).transpose(1, 0, 2).reshape(P, kt * QKVW)
    )
    biasqk = np.broadcast_to(b_sel[None, 0:512], (P, 512)).copy()

    # rope tables, natural layout per s-tile: [p, st*256 + jj]
    dims = np.arange(0, HD, 2, dtype=np.float64)
    invf = 1.0 / (THETA ** (dims / HD))  # [32]
    pos = np.arange(s, dtype=np.float64)
    ang = pos[:, None] * invf[None, :]  # [s, 32]
    c = np.cos(ang)
    sn = np.sin(ang)
    c2 = np.repeat(c, 2, axis=1)  # [s, 64]
    s2 = np.empty((s, HD))
    s2[:, 0::2] = -sn
    s2[:, 1::2] = sn
    c2h = np.tile(c2, (1, HPC))  # [s, 256]
    s2h = np.tile(s2, (1, HPC))
    ropec = np.ascontiguousarray(
        c2h.reshape(nt, P, 256).transpose(1, 0, 2).reshape(P, nt * 256)
    )
    ropes = np.ascontiguousarray(
        s2h.reshape(nt, P, 256).transpose(1, 0, 2).reshape(P, nt * 256)
    )

    trimask = np.triu(np.ones((P, P), dtype=np.float32))
    ident = np.eye(P, dtype=np.float32)

    # wo[kk, p2*D + n] = w_out[n, gh*64 + kk%64], gh = heads[2*p2 + kk//64]
    wo = np.empty((P, NPAIR * D), dtype=np.float32)
    for p2 in range(NPAIR):
        for half in range(2):
            gh = heads[2 * p2 + half]
            wo[half * 64 : (half + 1) * 64, p2 * D : (p2 + 1) * D] = w_out[
                :, gh * HD : (gh + 1) * HD
            ].T
    return {
        "xt": _to_pd(xt, mm_fast),
        "wt": _to_pd(wt, mm_fast),
        "biasqk": _to_pd(biasqk, mm_fast),
        "ropec": _to_pd(ropec, mm_fast),
        "ropes": _to_pd(ropes, mm_fast),
        "trimask": _to_pd(trimask, mm_fast),
        "ident": _to_pd(ident, mm_fast),
        "wo": _to_pd(wo, mm_fast),
    }


def kernel(x, w_qkv, b_qkv, w_out, b_out, mm_fast=True):
    global LAST_RESULTS
    x = np.asarray(x, dtype=np.float32)
    w_qkv = np.asarray(w_qkv, dtype=np.float32)
    b_qkv = np.asarray(b_qkv, dtype=np.float32)
    w_out = np.asarray(w_out, dtype=np.float32)
    b_out = np.asarray(b_out, dtype=np.float32)

    nc = get_program(mm_fast=mm_fast)
    in_maps = [
        prep_core_inputs(x, w_qkv, b_qkv, w_out, core, mm_fast=mm_fast)
        for core in range(NCORES)
    ]
    res = bass_utils.run_bass_kernel_spmd(
        nc, in_maps, core_ids=list(range(NCORES)), trace=TRACE
    )
    LAST_RESULTS = res
    partials = [r["outp"] for r in res.results]
    # v-bias contribution is constant across s (sum_k attn = 1):
    bconst = b_out + b_qkv[2 * D : 3 * D] @ w_out.T
    out = np.stack(
        [
            partials[0] + partials[1] + partials[2] + partials[3],
            partials[4] + partials[5] + partials[6] + partials[7],
        ]
    )
    out = out + bconst[None, None, :]
    return out.astype(np.float32)



# revision 2
# speedup vs baseline: 1.3112x; 1.3112x over previous
# Multi-head attention (RoPE, causal) Trainium2 Bass kernel, v2.
# B=2, S=2048, D=1024, 16 heads, hd=64, fp32 I/O.
#
# Sharding: 32 (batch, head) units over 8 cores -> each core gets one batch
# and 4 heads. Each core computes its 4 heads' attention output and the
# partial out-projection (sum over its heads); the host sums the 4 partials
# per batch and adds the bias constant.
#
# v2 vs v1: Q^T/K^T are produced directly transposed by the QKV projection
# (W stationary, X^T streaming) so no PE transposes are needed; RoPE runs in
# the transposed [hd, s] layout using a host-side W-row permutation that
# makes the rotation partner swap a single DVE stream_shuffle (swap the
# 16-halves of each 32-partition block); score matmuls are K=64 row-packed
# pairs (partitions 0:64 / 64:128) that execute concurrently on the PE;
# diagonal score blocks are N-trimmed; A/B phases are interleaved per
# 512-chunk to keep the PE dense.
#
# Self-contained: all shapes/sharding hardcoded; no sibling imports.

import numpy as np

import concourse.bass as bass  # noqa: F401
import concourse.mybir as mybir
import concourse.tile as tile
from concourse import bacc, bass_utils

F32 = mybir.dt.float32
BF16 = mybir.dt.bfloat16
EXP = mybir.ActivationFunctionType.Exp
ADD = mybir.AluOpType.add
MULT = mybir.AluOpType.mult

B = 2
S = 2048
D = 1024
NHEADS = 16
HD = 64
HPC = 4  # heads per core
NCORES = 8
NPAIR = 2  # head pairs per core
P = 128
CH = 512  # q chunk
NCH = S // CH  # 4
NT = S // P  # 16
KT = D // P  # 8
THETA = 10000.0

# swap the 16-halves of each 32-partition block (RoPE partner swap)
SWAP_MASK = list(range(16, 32)) + list(range(0, 16))

# module-level knobs for test harness
TRACE = False
LAST_RESULTS = None

_PROGRAM_CACHE = {}


def build_program():
    nc = bacc.Bacc(
        "TRN2", target_bir_lowering=False, debug=False, enable_asserts=False
    )

    # ---- DRAM I/O ----
    xt_d = nc.dram_tensor("xt", [P, KT * S], BF16, kind="ExternalInput").ap()
    wq_d = nc.dram_tensor("wq", [P, NPAIR * KT * P], BF16, kind="ExternalInput").ap()
    wk_d = nc.dram_tensor("wk", [P, NPAIR * KT * P], BF16, kind="ExternalInput").ap()
    wv_d = nc.dram_tensor("wv", [P, KT * 256], BF16, kind="ExternalInput").ap()
    bqk_d = nc.dram_tensor("bqk", [P, 4], F32, kind="ExternalInput").ap()
    ropec_d = nc.dram_tensor("ropec", [P, S], BF16, kind="ExternalInput").ap()
    ropes_d = nc.dram_tensor("ropes", [P, S], BF16, kind="ExternalInput").ap()
    trimask_d = nc.dram_tensor("trimask", [P, P], BF16, kind="ExternalInput").ap()
    wo_d = nc.dram_tensor("wo", [P, NPAIR * D], BF16, kind="ExternalInput").ap()
    out_d = nc.dram_tensor("outp", [S, D], F32, kind="ExternalOutput").ap()

    from contextlib import ExitStack

    with tile.TileContext(nc) as tc, ExitStack() as ctx:
        const = ctx.enter_context(tc.tile_pool(name="const", bufs=1))

        # weights for Q^T/K^T: lhsT tiles [128 (D-chunk k), 128 (pack rows)]
        wq_sb = const.tile([P, NPAIR * KT * P], BF16)
        wk_sb = const.tile([P, NPAIR * KT * P], BF16)
        wv_sb = const.tile([P, KT * 256], BF16)
        bqk_sb = const.tile([P, 4], F32)
        ropec_sb = const.tile([P, S], BF16)
        ropes_sb = const.tile([P, S], BF16)
        trimask_sb = const.tile([P, P], BF16)
        wo_sb = const.tile([P, NPAIR * D], BF16)
        # x^T, entire input: [128 (D-chunk k), k*S + t]
        xt_sb = const.tile([P, KT * S], BF16)

        # const loads: split across the two HWDGE rings; first-needed first.
        # sync ring: wq, then x chunk 0 k-slices, then the rest of x
        nc.sync.dma_start(wq_sb[:], wq_d[:])
        for c in range(NCH):
            for k in range(KT):
                sl = slice(k * S + c * CH, k * S + (c + 1) * CH)
                nc.sync.dma_start(xt_sb[:, sl], xt_d[:, sl])
        # scalar/ACT ring: everything else
        nc.scalar.dma_start(bqk_sb[:], bqk_d[:])
        nc.scalar.dma_start(ropec_sb[:], ropec_d[:])
        nc.scalar.dma_start(ropes_sb[:], ropes_d[:])
        nc.scalar.dma_start(wk_sb[:], wk_d[:])
        nc.scalar.dma_start(wv_sb[:], wv_d[:])
        nc.scalar.dma_start(trimask_sb[:], trimask_d[:])
        nc.scalar.dma_start(wo_sb[:], wo_d[:])

        # persistent activations
        # Q^T / K^T packs: [128 (2 heads x 64 permuted hd), s] per pack
        qt = const.tile([P, NPAIR * S], BF16)
        kt_sb = const.tile([P, NPAIR * S], BF16)
        # vone: per s-tile [128, 4*65]; per head 64 V cols + ones col
        vone = const.tile([P, NT * (HPC * 65)], BF16)
        # O^T packs (normalized in place): [128 (2 heads*hd), s] per pair
        otn2 = const.tile([P, NPAIR * S], BF16)

        vone_v = vone.rearrange("p (t h c) -> p t h c", t=NT, h=HPC)
        nc.gpsimd.memset(vone_v[:, :, :, 64], 1.0)

        # half-selector columns for the rank-1 denominator broadcast
        halfsel = const.tile([1, 256], BF16)
        nc.gpsimd.memset(halfsel[:, 0:64], 1.0)
        nc.gpsimd.memset(halfsel[:, 64:192], 0.0)
        nc.gpsimd.memset(halfsel[:, 192:256], 1.0)

        # softmax denominator collection: per (pair, J): [headA 512 | headB 512]
        dall = const.tile([1, NPAIR * NCH * 1024], F32)
        dallinv = const.tile([1, NPAIR * NCH * 1024], BF16)

        bpsum = ctx.enter_context(tc.tile_pool(name="bpsum", bufs=2, space="PSUM"))
        awork = ctx.enter_context(tc.tile_pool(name="awork", bufs=3))
        bwork = ctx.enter_context(tc.tile_pool(name="bwork", bufs=3))
        fxwork = ctx.enter_context(tc.tile_pool(name="fxwork", bufs=2))
        cwork = ctx.enter_context(tc.tile_pool(name="cwork", bufs=3))

        # ---------------- Phase A ----------------
        def emit_qkT(c, pk, w_sb, bias_col, dst):
            """One 512-col chunk of a Q^T or K^T pack, with bias + RoPE."""
            ps = bpsum.tile([P, CH], F32, name="psq", tag="big")
            for k in range(KT):
                nc.tensor.matmul(
                    ps[:],
                    lhsT=w_sb[:, (pk * KT + k) * P : (pk * KT + k + 1) * P],
                    rhs=xt_sb[:, k * S + c * CH : k * S + (c + 1) * CH],
                    start=(k == 0),
                    stop=(k == KT - 1),
                )
            # stage = ps + bias (per-partition)
            stage = awork.tile([P, CH], BF16, name="stage")
            nc.vector.tensor_scalar_add(stage[:], ps[:], bqk_sb[:, bias_col : bias_col + 1])
            # t1 = stage * S'' ; sw = shuffle(t1) ; rot = stage*C + sw
            t1 = awork.tile([P, CH], BF16, name="t1")
            nc.vector.tensor_mul(t1[:], stage[:], ropes_sb[:, c * CH : (c + 1) * CH])
            sw = awork.tile([P, CH], BF16, name="sw")
            nc.vector.stream_shuffle(sw[:], t1[:], SWAP_MASK)
            rot1 = awork.tile([P, CH], BF16, name="rot1")
            nc.vector.tensor_mul(rot1[:], stage[:], ropec_sb[:, c * CH : (c + 1) * CH])
            nc.vector.tensor_add(
                dst[:, pk * S + c * CH : pk * S + (c + 1) * CH], rot1[:], sw[:]
            )

        def emit_V(st):
            psv = bpsum.tile([P, 256], F32, name="psv", tag="acc")
            for k in range(KT):
                nc.tensor.matmul(
                    psv[:],
                    lhsT=xt_sb[:, k * S + st * P : k * S + (st + 1) * P],
                    rhs=wv_sb[:, k * 256 : (k + 1) * 256],
                    start=(k == 0),
                    stop=(k == KT - 1),
                )
            nc.scalar.copy(
                vone_v[:, st, :, 0:64],
                psv.rearrange("p (h c) -> p h c", h=HPC),
            )

        def emit_A(c):
            for pk in range(NPAIR):
                emit_qkT(c, pk, wq_sb, pk, qt)
            for pk in range(NPAIR):
                emit_qkT(c, pk, wk_sb, 2 + pk, kt_sb)
            for st in range(4 * c, 4 * c + 4):
                emit_V(st)

        # ---------------- Phase B: attention ----------------
        def emit_BJ(p, j):
            q_pack = qt[:, p * S : (p + 1) * S]
            k_pack = kt_sb[:, p * S : (p + 1) * S]
            ot2 = bpsum.tile([P, 1024], F32, name="ot2", tag="acc")
            mlast = 4 * j + 3
            for m in range(4 * j + 4):
                off = m * P - j * CH if m >= 4 * j else 0
                sc2 = bpsum.tile([P, 1024], F32, name="sc2", tag="big")
                nc.tensor.matmul(
                    sc2[:, off:CH],
                    lhsT=k_pack[0:64, m * P : (m + 1) * P],
                    rhs=q_pack[0:64, j * CH + off : (j + 1) * CH],
                )
                nc.tensor.matmul(
                    sc2[:, CH + off : 1024],
                    lhsT=k_pack[64:128, m * P : (m + 1) * P],
                    rhs=q_pack[64:128, j * CH + off : (j + 1) * CH],
                )
                at2 = bwork.tile([P, 1024], BF16, name="at2")
                if off > 0:
                    sc_v = sc2.rearrange("p (h q) -> p h q", h=2)
                    at_v = at2.rearrange("p (h q) -> p h q", h=2)
                    nc.scalar.activation(
                        at_v[:, :, off:CH], sc_v[:, :, off:CH], EXP, scale=0.125
                    )
                else:
                    nc.scalar.activation(at2[:], sc2[:], EXP, scale=0.125)
                if m >= 4 * j:
                    nc.vector.tensor_mul(
                        at2[:, off : off + P], at2[:, off : off + P], trimask_sb[:]
                    )
                    nc.vector.tensor_mul(
                        at2[:, CH + off : CH + off + P],
                        at2[:, CH + off : CH + off + P],
                        trimask_sb[:],
                    )
                nc.tensor.matmul(
                    ot2[0:65, off:CH],
                    lhsT=vone_v[:, m, 2 * p, :],
                    rhs=at2[:, off:CH],
                    start=(m == 0),
                    stop=(m == mlast),
                )
                nc.tensor.matmul(
                    ot2[0:65, CH + off : 1024],
                    lhsT=vone_v[:, m, 2 * p + 1, :],
                    rhs=at2[:, CH + off : 1024],
                    start=(m == 0),
                    stop=(m == mlast),
                )
            # ---- evict OT halves + denominators (per J) ----
            nc.vector.tensor_copy(
                otn2[0:64, p * S + j * CH : p * S + (j + 1) * CH], ot2[0:64, 0:CH]
            )
            stgB = fxwork.tile([64, CH], BF16, name="stgB")
            nc.vector.tensor_copy(stgB[:], ot2[0:64, CH:1024])
            nc.sync.dma_start(
                otn2[64:128, p * S + j * CH : p * S + (j + 1) * CH], stgB[:]
            )
            dslot = (p * NCH + j) * 1024
            nc.vector.tensor_copy(
                dall[0:1, dslot : dslot + 1024], ot2[64:65, 0:1024]
            )
            # reciprocal in partition-parallel layout
            dPj = fxwork.tile([P, 8], F32, name="dPj")
            nc.sync.dma_start(
                dPj[:],
                dall[0:1, dslot : dslot + 1024].rearrange("o (a b) -> o a b", a=P),
            )
            dPq = fxwork.tile([P, 8], F32, name="dPq")
            nc.vector.reciprocal(dPq[:], dPj[:])
            dPc = fxwork.tile([P, 8], BF16, name="dPc")
            with nc.allow_low_precision("softmax denominators"):
                nc.vector.tensor_copy(dPc[:], dPq[:])
            nc.sync.dma_start(
                dallinv[0:1, dslot : dslot + 1024].rearrange("o (a b) -> o a b", a=P),
                dPc[:],
            )

        def emit_final(p, j):
            dslot = (p * NCH + j) * 1024
            dvb = bpsum.tile([P, CH], F32, name="dvb", tag="acc")
            nc.tensor.matmul(
                dvb[:],
                lhsT=halfsel[0:1, 0:128],
                rhs=dallinv[0:1, dslot : dslot + CH],
                start=True,
                stop=False,
            )
            nc.tensor.matmul(
                dvb[:],
                lhsT=halfsel[0:1, 128:256],
                rhs=dallinv[0:1, dslot + CH : dslot + 1024],
                start=False,
                stop=True,
            )
            nc.vector.tensor_mul(
                otn2[:, p * S + j * CH : p * S + (j + 1) * CH],
                otn2[:, p * S + j * CH : p * S + (j + 1) * CH],
                dvb[:],
            )

        def emit_C(g):
            # out projection for q-tiles 4g..4g+3
            for qt_i in range(4 * g, 4 * g + 4):
                outsb = cwork.tile([P, D], F32, name="outsb")
                for dc in range(2):
                    pr = bpsum.tile([P, CH], F32, name="pr", tag="big")
                    for p in range(NPAIR):
                        nc.tensor.matmul(
                            pr[:],
                            lhsT=otn2[:, p * S + qt_i * P : p * S + (qt_i + 1) * P],
                            rhs=wo_sb[:, p * D + dc * CH : p * D + (dc + 1) * CH],
                            start=(p == 0),
                            stop=(p == NPAIR - 1),
                        )
                    if dc == 0:
                        nc.vector.tensor_copy(outsb[:, 0:CH], pr[:])
                    else:
                        nc.scalar.copy(outsb[:, CH:D], pr[:])
                nc.sync.dma_start(out_d[qt_i * P : (qt_i + 1) * P, :], outsb[:])

        # ---------------- schedule ----------------
        # A(0), A(1), then interleave B chunks one behind A; finals pipeline
        # one (p, j) behind; out-projection per chunk after pair-1 finals.
        emit_A(0)
        emit_A(1)
        pending = None

        def after_B(p, j):
            nonlocal pending
            if pending is not None:
                emit_final(*pending)
                if pending[0] == 1:
                    emit_C(pending[1])
            pending = (p, j)

        for j in range(NCH):
            for p in range(NPAIR):
                emit_BJ(p, j)
                after_B(p, j)
            if j + 2 < NCH:
                emit_A(j + 2)
        emit_final(*pending)
        emit_C(pending[1])

    nc.compile()
    return nc


def get_program():
    if "v2" not in _PROGRAM_CACHE:
        _PROGRAM_CACHE["v2"] = build_program()
    return _PROGRAM_CACHE["v2"]


def _bf16(a):
    import ml_dtypes

    return np.ascontiguousarray(a).astype(ml_dtypes.bfloat16)


def _rope_perm64():
    """Partition layout r -> original hd dim. Pairs are (r, r+16) within
    each 32-block: block b holds pairs 16b..16b+15; r%32<16 -> real (even
    dim), else imag (odd dim)."""
    perm = np.empty(HD, dtype=np.int64)
    for r in range(HD):
        blk, j = divmod(r, 32)
        pair = blk * 16 + (j % 16)
        imag = j // 16
        perm[r] = 2 * pair + imag
    return perm


_PERM64 = _rope_perm64()


def prep_core_inputs(x, w_qkv, b_qkv, w_out, core, xt_cache):
    b = core // 4
    heads = [(core % 4) * HPC + i for i in range(HPC)]

    if b not in xt_cache:
        xb = np.asarray(x[b])  # [S, D]
        xt = np.ascontiguousarray(
            xb.T.reshape(KT, P, S).transpose(1, 0, 2).reshape(P, KT * S)
        )
        xt_cache[b] = _bf16(xt)
    xt = xt_cache[b]

    # permuted row indices for Q/K packs: pack pk rows = heads 2pk, 2pk+1
    def pack_rows(section, pk):
        rows = []
        for half in range(2):
            h = heads[2 * pk + half]
            rows.extend(section * D + h * HD + _PERM64)
        return rows

    def wT_tiles(rows):
        w_sel = w_qkv[rows]  # [128, 1024]
        # lhsT[p, k*128 + c] = w_sel[c, k*128+p]
        return np.ascontiguousarray(
            w_sel.T.reshape(KT, P, P).transpose(1, 0, 2).reshape(P, KT * P)
        )

    wq = np.concatenate([wT_tiles(pack_rows(0, pk)) for pk in range(NPAIR)], axis=1)
    wk = np.concatenate([wT_tiles(pack_rows(1, pk)) for pk in range(NPAIR)], axis=1)

    # V natural: rhs tiles [128 (D-chunk), 256 (4 heads x 64 natural)]
    vrows = []
    for h in heads:
        vrows.extend(range(2 * D + h * HD, 2 * D + (h + 1) * HD))
    wv_sel = w_qkv[vrows]  # [256, 1024]
    wv = np.ascontiguousarray(
        wv_sel.T.reshape(KT, P, 256).transpose(1, 0, 2).reshape(P, KT * 256)
    )

    # bias columns [128, 4]: (Q pk0, Q pk1, K pk0, K pk1)
    bqk = np.empty((P, 4), dtype=np.float32)
    for qk in range(2):
        for pk in range(NPAIR):
            rows = pack_rows(qk, pk)
            bqk[:, qk * 2 + pk] = b_qkv[rows]

    # rope tables in permuted-partition layout [128, S]
    dims = np.arange(0, HD, 2, dtype=np.float64)
    invf = 1.0 / (THETA ** (dims / HD))  # [32] per pair index
    pos = np.arange(S, dtype=np.float64)
    r = np.arange(HD)
    blk, j = r // 32, r % 32
    pair = blk * 16 + (j % 16)
    is_imag = (j % 32) >= 16
    freq = invf[pair]  # [64]
    ang = pos[None, :] * freq[:, None]  # [64, S]
    c64 = np.cos(ang)
    # S[r] = -sin for real, +sin for imag; S''[r] = S[partner(r)] = flipped
    s64 = np.sin(ang) * np.where(is_imag, -1.0, 1.0)[:, None]
    ropec = np.tile(c64, (2, 1))  # [128, S]
    ropes = np.tile(s64, (2, 1))

    trimask = np.triu(np.ones((P, P), dtype=np.float32))

    # wo[kk, p2*D + n] = w_out[n, gh*64 + kk%64], gh = heads[2*p2 + kk//64]
    wo = np.empty((P, NPAIR * D), dtype=np.float32)
    for p2 in range(NPAIR):
        for half in range(2):
            gh = heads[2 * p2 + half]
            wo[half * 64 : (half + 1) * 64, p2 * D : (p2 + 1) * D] = w_out[
                :, gh * HD : (gh + 1) * HD
            ].T
    return {
        "xt": xt,
        "wq": _bf16(wq),
        "wk": _bf16(wk),
        "wv": _bf16(wv),
        "bqk": np.ascontiguousarray(bqk),
        "ropec": _bf16(ropec),
        "ropes": _bf16(ropes),
        "trimask": _bf16(trimask),
        "wo": _bf16(wo),
    }


def kernel(x, w_qkv, b_qkv, w_out, b_out):
    global LAST_RESULTS
    x = np.asarray(x, dtype=np.float32)
    w_qkv = np.asarray(w_qkv, dtype=np.float32)
    b_qkv = np.asarray(b_qkv, dtype=np.float32)
    w_out = np.asarray(w_out, dtype=np.float32)
    b_out = np.asarray(b_out, dtype=np.float32)

    nc = get_program()
    xt_cache = {}
    in_maps = [
        prep_core_inputs(x, w_qkv, b_qkv, w_out, core, xt_cache)
        for core in range(NCORES)
    ]
    res = bass_utils.run_bass_kernel_spmd(
        nc, in_maps, core_ids=list(range(NCORES)), trace=TRACE
    )
    LAST_RESULTS = res
    partials = [r["outp"] for r in res.results]
    # v-bias contribution is constant across s (sum_k attn = 1):
    bconst = b_out + b_qkv[2 * D : 3 * D] @ w_out.T
    out = np.stack(
        [
            partials[0] + partials[1] + partials[2] + partials[3],
            partials[4] + partials[5] + partials[6] + partials[7],
        ]
    )
    out = out + bconst[None, None, :]
    return out.astype(np.float32)


# revision 4
# speedup vs baseline: 1.4135x; 1.0780x over previous
# Multi-head attention (RoPE, causal) Trainium2 Bass kernel, v2.
# B=2, S=2048, D=1024, 16 heads, hd=64, fp32 I/O.
#
# Sharding: 32 (batch, head) units over 8 cores -> each core gets one batch
# and 4 heads. Each core computes its 4 heads' attention output and the
# partial out-projection (sum over its heads); the host sums the 4 partials
# per batch and adds the bias constant.
#
# v2 vs v1: Q^T/K^T are produced directly transposed by the QKV projection
# (W stationary, X^T streaming) so no PE transposes are needed; RoPE runs in
# the transposed [hd, s] layout using a host-side W-row permutation that
# makes the rotation partner swap a single DVE stream_shuffle (swap the
# 16-halves of each 32-partition block); score matmuls are K=64 row-packed
# pairs (partitions 0:64 / 64:128) that execute concurrently on the PE;
# diagonal score blocks are N-trimmed; A/B phases are interleaved per
# 512-chunk to keep the PE dense.
#
# Self-contained: all shapes/sharding hardcoded; no sibling imports.

import numpy as np

import concourse.bass as bass  # noqa: F401
import concourse.mybir as mybir
import concourse.tile as tile
from concourse import bacc, bass_utils

F32 = mybir.dt.float32
BF16 = mybir.dt.bfloat16
EXP = mybir.ActivationFunctionType.Exp
ADD = mybir.AluOpType.add
MULT = mybir.AluOpType.mult

B = 2
S = 2048
D = 1024
NHEADS = 16
HD = 64
HPC = 4  # heads per core
NCORES = 8
NPAIR = 2  # head pairs per core
P = 128
CH = 512  # q chunk
NCH = S // CH  # 4
NT = S // P  # 16
KT = D // P  # 8
THETA = 10000.0

# swap the 16-halves of each 32-partition block (RoPE partner swap)
SWAP_MASK = list(range(16, 32)) + list(range(0, 16))

# module-level knobs for test harness
TRACE = False
LAST_RESULTS = None

_PROGRAM_CACHE = {}


def build_program():
    nc = bacc.Bacc(
        "TRN2", target_bir_lowering=False, debug=False, enable_asserts=False
    )

    # ---- DRAM I/O ----
    xt_d = nc.dram_tensor("xt", [P, KT * S], BF16, kind="ExternalInput").ap()
    wq_d = nc.dram_tensor("wq", [P, NPAIR * KT * P], BF16, kind="ExternalInput").ap()
    wk_d = nc.dram_tensor("wk", [P, NPAIR * KT * P], BF16, kind="ExternalInput").ap()
    wv_d = nc.dram_tensor("wv", [P, KT * 256], BF16, kind="ExternalInput").ap()
    bqk_d = nc.dram_tensor("bqk", [P, 4], F32, kind="ExternalInput").ap()
    ropec_d = nc.dram_tensor("ropec", [P, S], BF16, kind="ExternalInput").ap()
    ropes_d = nc.dram_tensor("ropes", [P, S], BF16, kind="ExternalInput").ap()
    trimask_d = nc.dram_tensor("trimask", [P, P], BF16, kind="ExternalInput").ap()
    wo_d = nc.dram_tensor("wo", [P, NPAIR * D], BF16, kind="ExternalInput").ap()
    out_d = nc.dram_tensor("outp", [S, D], F32, kind="ExternalOutput").ap()

    from contextlib import ExitStack

    with tile.TileContext(nc) as tc, ExitStack() as ctx:
        const = ctx.enter_context(tc.tile_pool(name="const", bufs=1))

        # weights for Q^T/K^T: lhsT tiles [128 (D-chunk k), 128 (pack rows)]
        wq_sb = const.tile([P, NPAIR * KT * P], BF16)
        wk_sb = const.tile([P, NPAIR * KT * P], BF16)
        wv_sb = const.tile([P, KT * 256], BF16)
        bqk_sb = const.tile([P, 4], F32)
        ropec_sb = const.tile([P, S], BF16)
        ropes_sb = const.tile([P, S], BF16)
        trimask_sb = const.tile([P, P], BF16)
        wo_sb = const.tile([P, NPAIR * D], BF16)
        # x^T, entire input: [128 (D-chunk k), k*S + t]
        xt_sb = const.tile([P, KT * S], BF16)

        # const loads: split across the two HWDGE rings; first-needed first.
        # sync ring: wq pack0, x chunk 0 k-slices, wq pack1, rest of x
        nc.sync.dma_start(wq_sb[:, 0 : KT * P], wq_d[:, 0 : KT * P])
        for k in range(KT):
            sl = slice(k * S, k * S + CH)
            nc.sync.dma_start(xt_sb[:, sl], xt_d[:, sl])
        nc.sync.dma_start(wq_sb[:, KT * P :], wq_d[:, KT * P :])
        for c in range(1, NCH):
            for k in range(KT):
                sl = slice(k * S + c * CH, k * S + (c + 1) * CH)
                nc.sync.dma_start(xt_sb[:, sl], xt_d[:, sl])
        # scalar/ACT ring: everything else
        nc.scalar.dma_start(wk_sb[:], wk_d[:])
        nc.scalar.dma_start(bqk_sb[:], bqk_d[:])
        nc.scalar.dma_start(ropec_sb[:], ropec_d[:])
        nc.scalar.dma_start(ropes_sb[:], ropes_d[:])
        nc.scalar.dma_start(wv_sb[:], wv_d[:])
        nc.scalar.dma_start(trimask_sb[:], trimask_d[:])
        nc.scalar.dma_start(wo_sb[:], wo_d[:])

        # persistent activations
        # Q^T / K^T packs: [128 (2 heads x 64 permuted hd), s] per pack
        qt = const.tile([P, NPAIR * S], BF16)
        kt_sb = const.tile([P, NPAIR * S], BF16)
        # vone: per s-tile [128, 4*65]; per head 64 V cols + ones col
        vone = const.tile([P, NT * (HPC * 65)], BF16)
        # O^T packs (normalized in place): [128 (2 heads*hd), s] per pair
        otn2 = const.tile([P, NPAIR * S], BF16)

        vone_v = vone.rearrange("p (t h c) -> p t h c", t=NT, h=HPC)
        nc.gpsimd.memset(vone_v[:, :, :, 64], 1.0)

        # half-selector columns for the rank-1 denominator broadcast
        halfsel = const.tile([1, 256], BF16)
        nc.gpsimd.memset(halfsel[:, 0:64], 1.0)
        nc.gpsimd.memset(halfsel[:, 64:192], 0.0)
        nc.gpsimd.memset(halfsel[:, 192:256], 1.0)

        # softmax denominator collection: per (pair, J): [headA 512 | headB 512]
        dall = const.tile([1, NPAIR * NCH * 1024], F32)
        dallinv = const.tile([1, NPAIR * NCH * 1024], BF16)

        bpsum = ctx.enter_context(tc.tile_pool(name="bpsum", bufs=2, space="PSUM"))
        opsum = ctx.enter_context(tc.tile_pool(name="opsum", bufs=1, space="PSUM"))
        awork = ctx.enter_context(tc.tile_pool(name="awork", bufs=3))
        bwork = ctx.enter_context(tc.tile_pool(name="bwork", bufs=3))
        fxwork = ctx.enter_context(tc.tile_pool(name="fxwork", bufs=2))
        cwork = ctx.enter_context(tc.tile_pool(name="cwork", bufs=3))

        # ---------------- Phase A ----------------
        def emit_qkT(c, pk, w_sb, bias_col, dst):
            """One 512-col chunk of a Q^T or K^T pack, with bias + RoPE."""
            ps = bpsum.tile([P, CH], F32, name="psq", tag="proj")
            for k in range(KT):
                nc.tensor.matmul(
                    ps[:],
                    lhsT=w_sb[:, (pk * KT + k) * P : (pk * KT + k + 1) * P],
                    rhs=xt_sb[:, k * S + c * CH : k * S + (c + 1) * CH],
                    start=(k == 0),
                    stop=(k == KT - 1),
                )
            # stage = ps + bias (per-partition)
            stage = awork.tile([P, CH], BF16, name="stage")
            nc.vector.tensor_scalar_add(stage[:], ps[:], bqk_sb[:, bias_col : bias_col + 1])
            # t1 = stage * S'' ; sw = shuffle(t1) ; rot = stage*C + sw
            t1 = awork.tile([P, CH], BF16, name="t1")
            nc.vector.tensor_mul(t1[:], stage[:], ropes_sb[:, c * CH : (c + 1) * CH])
            sw = awork.tile([P, CH], BF16, name="sw")
            nc.vector.stream_shuffle(sw[:], t1[:], SWAP_MASK)
            rot1 = awork.tile([P, CH], BF16, name="rot1")
            nc.vector.tensor_mul(rot1[:], stage[:], ropec_sb[:, c * CH : (c + 1) * CH])
            nc.vector.tensor_add(
                dst[:, pk * S + c * CH : pk * S + (c + 1) * CH], rot1[:], sw[:]
            )

        def emit_V(st):
            psv = bpsum.tile([P, 256], F32, name="psv", tag="proj")
            for k in range(KT):
                nc.tensor.matmul(
                    psv[:],
                    lhsT=xt_sb[:, k * S + st * P : k * S + (st + 1) * P],
                    rhs=wv_sb[:, k * 256 : (k + 1) * 256],
                    start=(k == 0),
                    stop=(k == KT - 1),
                )
            nc.scalar.copy(
                vone_v[:, st, :, 0:64],
                psv.rearrange("p (h c) -> p h c", h=HPC),
            )

        # ---------------- Phase B: attention ----------------
        def emit_BJ(p, j):
            q_pack = qt[:, p * S : (p + 1) * S]
            k_pack = kt_sb[:, p * S : (p + 1) * S]
            ot2 = opsum.tile([P, 1024], F32, name="ot2", tag="acc")
            mlast = 4 * j + 3
            for m in range(4 * j + 4):
                off = m * P - j * CH if m >= 4 * j else 0
                sc2 = bpsum.tile([P, 1024], F32, name="sc2", tag="sc")
                nc.tensor.matmul(
                    sc2[:, off:CH],
                    lhsT=k_pack[0:64, m * P : (m + 1) * P],
                    rhs=q_pack[0:64, j * CH + off : (j + 1) * CH],
                )
                nc.tensor.matmul(
                    sc2[:, CH + off : 1024],
                    lhsT=k_pack[64:128, m * P : (m + 1) * P],
                    rhs=q_pack[64:128, j * CH + off : (j + 1) * CH],
                )
                at2 = bwork.tile([P, 1024], BF16, name="at2")
                if off > 0:
                    sc_v = sc2.rearrange("p (h q) -> p h q", h=2)
                    at_v = at2.rearrange("p (h q) -> p h q", h=2)
                    nc.scalar.activation(
                        at_v[:, :, off:CH], sc_v[:, :, off:CH], EXP, scale=0.125
                    )
                else:
                    nc.scalar.activation(at2[:], sc2[:], EXP, scale=0.125)
                if m >= 4 * j:
                    nc.vector.tensor_mul(
                        at2[:, off : off + P], at2[:, off : off + P], trimask_sb[:]
                    )
                    nc.vector.tensor_mul(
                        at2[:, CH + off : CH + off + P],
                        at2[:, CH + off : CH + off + P],
                        trimask_sb[:],
                    )
                nc.tensor.matmul(
                    ot2[0:65, off:CH],
                    lhsT=vone_v[:, m, 2 * p, :],
                    rhs=at2[:, off:CH],
                    start=(m == 0),
                    stop=(m == mlast),
                )
                nc.tensor.matmul(
                    ot2[0:65, CH + off : 1024],
                    lhsT=vone_v[:, m, 2 * p + 1, :],
                    rhs=at2[:, CH + off : 1024],
                    start=(m == 0),
                    stop=(m == mlast),
                )
            # ---- evict OT halves + denominators (per J) ----
            nc.vector.tensor_copy(
                otn2[0:64, p * S + j * CH : p * S + (j + 1) * CH], ot2[0:64, 0:CH]
            )
            stgB = fxwork.tile([64, CH], BF16, name="stgB")
            nc.vector.tensor_copy(stgB[:], ot2[0:64, CH:1024])
            nc.sync.dma_start(
                otn2[64:128, p * S + j * CH : p * S + (j + 1) * CH], stgB[:]
            )
            dslot = (p * NCH + j) * 1024
            nc.vector.tensor_copy(
                dall[0:1, dslot : dslot + 1024], ot2[64:65, 0:1024]
            )
            # reciprocal in partition-parallel layout
            dPj = fxwork.tile([P, 8], F32, name="dPj")
            nc.sync.dma_start(
                dPj[:],
                dall[0:1, dslot : dslot + 1024].rearrange("o (a b) -> o a b", a=P),
            )
            dPq = fxwork.tile([P, 8], F32, name="dPq")
            nc.vector.reciprocal(dPq[:], dPj[:])
            dPc = fxwork.tile([P, 8], BF16, name="dPc")
            with nc.allow_low_precision("softmax denominators"):
                nc.vector.tensor_copy(dPc[:], dPq[:])
            nc.sync.dma_start(
                dallinv[0:1, dslot : dslot + 1024].rearrange("o (a b) -> o a b", a=P),
                dPc[:],
            )

        def emit_final(p, j):
            dslot = (p * NCH + j) * 1024
            dvb = bpsum.tile([P, CH], F32, name="dvb", tag="proj")
            nc.tensor.matmul(
                dvb[:],
                lhsT=halfsel[0:1, 0:128],
                rhs=dallinv[0:1, dslot : dslot + CH],
                start=True,
                stop=False,
            )
            nc.tensor.matmul(
                dvb[:],
                lhsT=halfsel[0:1, 128:256],
                rhs=dallinv[0:1, dslot + CH : dslot + 1024],
                start=False,
                stop=True,
            )
            nc.vector.tensor_mul(
                otn2[:, p * S + j * CH : p * S + (j + 1) * CH],
                otn2[:, p * S + j * CH : p * S + (j + 1) * CH],
                dvb[:],
            )

        def emit_C(g):
            # out projection for q-tiles 4g..4g+3
            for qt_i in range(4 * g, 4 * g + 4):
                outsb = cwork.tile([P, D], F32, name="outsb")
                for dc in range(2):
                    pr = bpsum.tile([P, CH], F32, name="pr", tag="proj")
                    for p in range(NPAIR):
                        nc.tensor.matmul(
                            pr[:],
                            lhsT=otn2[:, p * S + qt_i * P : p * S + (qt_i + 1) * P],
                            rhs=wo_sb[:, p * D + dc * CH : p * D + (dc + 1) * CH],
                            start=(p == 0),
                            stop=(p == NPAIR - 1),
                        )
                    if dc == 0:
                        nc.vector.tensor_copy(outsb[:, 0:CH], pr[:])
                    else:
                        nc.scalar.copy(outsb[:, CH:D], pr[:])
                nc.sync.dma_start(out_d[qt_i * P : (qt_i + 1) * P, :], outsb[:])

        # ---------------- schedule ----------------
        # A(0) split around B(0,0)'s needs; A(j+1) pieces interleave inside
        # the j-loop so PE fills ACT-bound B segments; finals pipeline one
        # (p, j) behind; out-projection per chunk after pair-1 finals.
        emit_qkT(0, 0, wq_sb, 0, qt)
        emit_qkT(0, 0, wk_sb, 2, kt_sb)
        for st in range(4):
            emit_V(st)
        emit_qkT(0, 1, wq_sb, 1, qt)
        emit_qkT(0, 1, wk_sb, 3, kt_sb)
        pending = None

        def after_B(p, j):
            nonlocal pending
            if pending is not None:
                emit_final(*pending)
                if pending[0] == 1:
                    emit_C(pending[1])
            pending = (p, j)

        for j in range(NCH):
            emit_BJ(0, j)
            after_B(0, j)
            if j + 1 < NCH:
                emit_qkT(j + 1, 0, wq_sb, 0, qt)
                emit_qkT(j + 1, 0, wk_sb, 2, kt_sb)
                for st in range(4 * j + 4, 4 * j + 8):
                    emit_V(st)
            emit_BJ(1, j)
            after_B(1, j)
            if j + 1 < NCH:
                emit_qkT(j + 1, 1, wq_sb, 1, qt)
                emit_qkT(j + 1, 1, wk_sb, 3, kt_sb)
        emit_final(*pending)
        emit_C(pending[1])

    nc.compile()
    return nc


def get_program():
    if "v2" not in _PROGRAM_CACHE:
        _PROGRAM_CACHE["v2"] = build_program()
    return _PROGRAM_CACHE["v2"]


def _bf16(a):
    import ml_dtypes

    return np.ascontiguousarray(a).astype(ml_dtypes.bfloat16)


def _rope_perm64():
    """Partition layout r -> original hd dim. Pairs are (r, r+16) within
    each 32-block: block b holds pairs 16b..16b+15; r%32<16 -> real (even
    dim), else imag (odd dim)."""
    perm = np.empty(HD, dtype=np.int64)
    for r in range(HD):
        blk, j = divmod(r, 32)
        pair = blk * 16 + (j % 16)
        imag = j // 16
        perm[r] = 2 * pair + imag
    return perm


_PERM64 = _rope_perm64()


def prep_core_inputs(x, w_qkv, b_qkv, w_out, core, xt_cache):
    b = core // 4
    heads = [(core % 4) * HPC + i for i in range(HPC)]

    if b not in xt_cache:
        xb = np.asarray(x[b])  # [S, D]
        xt = np.ascontiguousarray(
            xb.T.reshape(KT, P, S).transpose(1, 0, 2).reshape(P, KT * S)
        )
        xt_cache[b] = _bf16(xt)
    xt = xt_cache[b]

    # permuted row indices for Q/K packs: pack pk rows = heads 2pk, 2pk+1
    def pack_rows(section, pk):
        rows = []
        for half in range(2):
            h = heads[2 * pk + half]
            rows.extend(section * D + h * HD + _PERM64)
        return rows

    def wT_tiles(rows):
        w_sel = w_qkv[rows]  # [128, 1024]
        # lhsT[p, k*128 + c] = w_sel[c, k*128+p]
        return np.ascontiguousarray(
            w_sel.T.reshape(KT, P, P).transpose(1, 0, 2).reshape(P, KT * P)
        )

    wq = np.concatenate([wT_tiles(pack_rows(0, pk)) for pk in range(NPAIR)], axis=1)
    wk = np.concatenate([wT_tiles(pack_rows(1, pk)) for pk in range(NPAIR)], axis=1)

    # V natural: rhs tiles [128 (D-chunk), 256 (4 heads x 64 natural)]
    vrows = []
    for h in heads:
        vrows.extend(range(2 * D + h * HD, 2 * D + (h + 1) * HD))
    wv_sel = w_qkv[vrows]  # [256, 1024]
    wv = np.ascontiguousarray(
        wv_sel.T.reshape(KT, P, 256).transpose(1, 0, 2).reshape(P, KT * 256)
    )

    # bias columns [128, 4]: (Q pk0, Q pk1, K pk0, K pk1)
    bqk = np.empty((P, 4), dtype=np.float32)
    for qk in range(2):
        for pk in range(NPAIR):
            rows = pack_rows(qk, pk)
            bqk[:, qk * 2 + pk] = b_qkv[rows]

    # rope tables in permuted-partition layout [128, S]
    dims = np.arange(0, HD, 2, dtype=np.float64)
    invf = 1.0 / (THETA ** (dims / HD))  # [32] per pair index
    pos = np.arange(S, dtype=np.float64)
    r = np.arange(HD)
    blk, j = r // 32, r % 32
    pair = blk * 16 + (j % 16)
    is_imag = (j % 32) >= 16
    freq = invf[pair]  # [64]
    ang = pos[None, :] * freq[:, None]  # [64, S]
    c64 = np.cos(ang)
    # S[r] = -sin for real, +sin for imag; S''[r] = S[partner(r)] = flipped
    s64 = np.sin(ang) * np.where(is_imag, -1.0, 1.0)[:, None]
    ropec = np.tile(c64, (2, 1))  # [128, S]
    ropes = np.tile(s64, (2, 1))

    trimask = np.triu(np.ones((P, P), dtype=np.float32))

    # wo[kk, p2*D + n] = w_out[n, gh*64 + kk%64], gh = heads[2*p2 + kk//64]
    wo = np.empty((P, NPAIR * D), dtype=np.float32)
    for p2 in range(NPAIR):
        for half in range(2):
            gh = heads[2 * p2 + half]
            wo[half * 64 : (half + 1) * 64, p2 * D : (p2 + 1) * D] = w_out[
                :, gh * HD : (gh + 1) * HD
            ].T
    return {
        "xt": xt,
        "wq": _bf16(wq),
        "wk": _bf16(wk),
        "wv": _bf16(wv),
        "bqk": np.ascontiguousarray(bqk),
        "ropec": _bf16(ropec),
        "ropes": _bf16(ropes),
        "trimask": _bf16(trimask),
        "wo": _bf16(wo),
    }


def kernel(x, w_qkv, b_qkv, w_out, b_out):
    global LAST_RESULTS
    x = np.asarray(x, dtype=np.float32)
    w_qkv = np.asarray(w_qkv, dtype=np.float32)
    b_qkv = np.asarray(b_qkv, dtype=np.float32)
    w_out = np.asarray(w_out, dtype=np.float32)
    b_out = np.asarray(b_out, dtype=np.float32)

    nc = get_program()
    xt_cache = {}
    in_maps = [
        prep_core_inputs(x, w_qkv, b_qkv, w_out, core, xt_cache)
        for core in range(NCORES)
    ]
    res = bass_utils.run_bass_kernel_spmd(
        nc, in_maps, core_ids=list(range(NCORES)), trace=TRACE
    )
    LAST_RESULTS = res
    partials = [r["outp"] for r in res.results]
    # v-bias contribution is constant across s (sum_k attn = 1):
    bconst = b_out + b_qkv[2 * D : 3 * D] @ w_out.T
    out = np.stack(
        [
            partials[0] + partials[1] + partials[2] + partials[3],
            partials[4] + partials[5] + partials[6] + partials[7],
        ]
    )
    out = out + bconst[None, None, :]
    return out.astype(np.float32)


# revision 8
# speedup vs baseline: 1.4413x; 1.0197x over previous
# Multi-head attention (RoPE, causal) Trainium2 Bass kernel, v2.
# B=2, S=2048, D=1024, 16 heads, hd=64, fp32 I/O.
#
# Sharding: 32 (batch, head) units over 8 cores -> each core gets one batch
# and 4 heads. Each core computes its 4 heads' attention output and the
# partial out-projection (sum over its heads); the host sums the 4 partials
# per batch and adds the bias constant.
#
# v2 vs v1: Q^T/K^T are produced directly transposed by the QKV projection
# (W stationary, X^T streaming) so no PE transposes are needed; RoPE runs in
# the transposed [hd, s] layout using a host-side W-row permutation that
# makes the rotation partner swap a single DVE stream_shuffle (swap the
# 16-halves of each 32-partition block); score matmuls are K=64 row-packed
# pairs (partitions 0:64 / 64:128) that execute concurrently on the PE;
# diagonal score blocks are N-trimmed; A/B phases are interleaved per
# 512-chunk to keep the PE dense.
#
# Self-contained: all shapes/sharding hardcoded; no sibling imports.

import numpy as np

import concourse.bass as bass  # noqa: F401
import concourse.mybir as mybir
import concourse.tile as tile
from concourse import bacc, bass_utils

F32 = mybir.dt.float32
BF16 = mybir.dt.bfloat16
EXP = mybir.ActivationFunctionType.Exp
ADD = mybir.AluOpType.add
MULT = mybir.AluOpType.mult

B = 2
S = 2048
D = 1024
NHEADS = 16
HD = 64
HPC = 4  # heads per core
NCORES = 8
NPAIR = 2  # head pairs per core
P = 128
CH = 512  # q chunk
NCH = S // CH  # 4
NT = S // P  # 16
KT = D // P  # 8
THETA = 10000.0

# swap the 16-halves of each 32-partition block (RoPE partner swap)
SWAP_MASK = list(range(16, 32)) + list(range(0, 16))

# module-level knobs for test harness
TRACE = False
LAST_RESULTS = None

_PROGRAM_CACHE = {}


def build_program():
    nc = bacc.Bacc(
        "TRN2", target_bir_lowering=False, debug=False, enable_asserts=False
    )

    # ---- DRAM I/O ----
    xt_d = nc.dram_tensor("xt", [P, KT * S], BF16, kind="ExternalInput").ap()
    wq_d = nc.dram_tensor("wq", [P, NPAIR * KT * P], BF16, kind="ExternalInput").ap()
    wk_d = nc.dram_tensor("wk", [P, NPAIR * KT * P], BF16, kind="ExternalInput").ap()
    wv_d = nc.dram_tensor("wv", [P, KT * 256], BF16, kind="ExternalInput").ap()
    bqk_d = nc.dram_tensor("bqk", [P, 4], F32, kind="ExternalInput").ap()
    ropec_d = nc.dram_tensor("ropec", [P, S], BF16, kind="ExternalInput").ap()
    ropes_d = nc.dram_tensor("ropes", [P, S], BF16, kind="ExternalInput").ap()
    trimask_d = nc.dram_tensor("trimask", [P, P], BF16, kind="ExternalInput").ap()
    wo_d = nc.dram_tensor("wo", [P, NPAIR * D], BF16, kind="ExternalInput").ap()
    out_d = nc.dram_tensor("outp", [S, D], F32, kind="ExternalOutput").ap()

    from contextlib import ExitStack

    with tile.TileContext(nc) as tc, ExitStack() as ctx:
        const = ctx.enter_context(tc.tile_pool(name="const", bufs=1))

        # weights for Q^T/K^T: lhsT tiles [128 (D-chunk k), 128 (pack rows)]
        wq_sb = const.tile([P, NPAIR * KT * P], BF16)
        wk_sb = const.tile([P, NPAIR * KT * P], BF16)
        wv_sb = const.tile([P, KT * 256], BF16)
        bqk_sb = const.tile([P, 4], F32)
        ropec_sb = const.tile([P, S], BF16)
        ropes_sb = const.tile([P, S], BF16)
        trimask_sb = const.tile([P, P], BF16)
        wo_sb = const.tile([P, NPAIR * D], BF16)
        # x^T, entire input: [128 (D-chunk k), k*S + t]
        xt_sb = const.tile([P, KT * S], BF16)

        # const loads: split across the two HWDGE rings; first-needed first.
        # sync ring: wq pack0, x chunk 0 k-slices, wq pack1, rest of x
        nc.sync.dma_start(wq_sb[:, 0 : KT * P], wq_d[:, 0 : KT * P])
        for k in range(KT):
            sl = slice(k * S, k * S + CH)
            nc.sync.dma_start(xt_sb[:, sl], xt_d[:, sl])
        nc.sync.dma_start(wq_sb[:, KT * P :], wq_d[:, KT * P :])
        for c in range(1, NCH):
            for k in range(KT):
                sl = slice(k * S + c * CH, k * S + (c + 1) * CH)
                nc.sync.dma_start(xt_sb[:, sl], xt_d[:, sl])
        # scalar/ACT ring: everything else
        nc.scalar.dma_start(wk_sb[:], wk_d[:])
        nc.scalar.dma_start(bqk_sb[:], bqk_d[:])
        nc.scalar.dma_start(ropec_sb[:], ropec_d[:])
        nc.scalar.dma_start(ropes_sb[:], ropes_d[:])
        nc.scalar.dma_start(wv_sb[:], wv_d[:])
        nc.scalar.dma_start(trimask_sb[:], trimask_d[:])
        nc.scalar.dma_start(wo_sb[:], wo_d[:])

        # persistent activations
        # Q^T / K^T packs: [128 (2 heads x 64 permuted hd), s] per pack
        qt = const.tile([P, NPAIR * S], BF16)
        kt_sb = const.tile([P, NPAIR * S], BF16)
        # vone: per s-tile [128, 4*65]; per head 64 V cols + ones col
        vone = const.tile([P, NT * (HPC * 65)], BF16)
        # O^T packs (normalized in place): [128 (2 heads*hd), s] per pair
        otn2 = const.tile([P, NPAIR * S], BF16)

        vone_v = vone.rearrange("p (t h c) -> p t h c", t=NT, h=HPC)
        nc.gpsimd.memset(vone_v[:, :, :, 64], 1.0)

        # half-selector columns for the rank-1 denominator broadcast
        halfsel = const.tile([1, 256], BF16)
        nc.gpsimd.memset(halfsel[:, 0:64], 1.0)
        nc.gpsimd.memset(halfsel[:, 64:192], 0.0)
        nc.gpsimd.memset(halfsel[:, 192:256], 1.0)

        # softmax denominator collection: per (pair, J): [headA 512 | headB 512]
        dall = const.tile([1, NPAIR * NCH * 1024], F32)
        dallinv = const.tile([1, NPAIR * NCH * 1024], BF16)

        bpsum = ctx.enter_context(tc.tile_pool(name="bpsum", bufs=2, space="PSUM"))
        opsum = ctx.enter_context(tc.tile_pool(name="opsum", bufs=1, space="PSUM"))
        awork = ctx.enter_context(tc.tile_pool(name="awork", bufs=3))
        bwork = ctx.enter_context(tc.tile_pool(name="bwork", bufs=4))
        fxwork = ctx.enter_context(tc.tile_pool(name="fxwork", bufs=2))
        cwork = ctx.enter_context(tc.tile_pool(name="cwork", bufs=3))

        # ---------------- Phase A ----------------
        def emit_qkT(c, pk, w_sb, bias_col, dst):
            """One 512-col chunk of a Q^T or K^T pack, with bias + RoPE."""
            ps = bpsum.tile([P, CH], F32, name="psq", tag="proj")
            for k in range(KT):
                nc.tensor.matmul(
                    ps[:],
                    lhsT=w_sb[:, (pk * KT + k) * P : (pk * KT + k + 1) * P],
                    rhs=xt_sb[:, k * S + c * CH : k * S + (c + 1) * CH],
                    start=(k == 0),
                    stop=(k == KT - 1),
                )
            # stage = ps + bias (per-partition)
            stage = awork.tile([P, CH], BF16, name="stage")
            nc.vector.tensor_scalar_add(stage[:], ps[:], bqk_sb[:, bias_col : bias_col + 1])
            # t1 = stage * S'' ; sw = shuffle(t1) ; rot = stage*C + sw
            t1 = awork.tile([P, CH], BF16, name="t1")
            nc.vector.tensor_mul(t1[:], stage[:], ropes_sb[:, c * CH : (c + 1) * CH])
            sw = awork.tile([P, CH], BF16, name="sw")
            nc.vector.stream_shuffle(sw[:], t1[:], SWAP_MASK)
            rot1 = awork.tile([P, CH], BF16, name="rot1")
            nc.vector.tensor_mul(rot1[:], stage[:], ropec_sb[:, c * CH : (c + 1) * CH])
            nc.vector.tensor_add(
                dst[:, pk * S + c * CH : pk * S + (c + 1) * CH], rot1[:], sw[:]
            )

        def emit_V(st):
            psv = bpsum.tile([P, 256], F32, name="psv", tag="proj")
            for k in range(KT):
                nc.tensor.matmul(
                    psv[:],
                    lhsT=xt_sb[:, k * S + st * P : k * S + (st + 1) * P],
                    rhs=wv_sb[:, k * 256 : (k + 1) * 256],
                    start=(k == 0),
                    stop=(k == KT - 1),
                )
            nc.scalar.copy(
                vone_v[:, st, :, 0:64],
                psv.rearrange("p (h c) -> p h c", h=HPC),
            )

        # ---------------- Phase B: attention ----------------
        def emit_BJ(p, j):
            q_pack = qt[:, p * S : (p + 1) * S]
            k_pack = kt_sb[:, p * S : (p + 1) * S]
            ot2 = opsum.tile([P, 1024], F32, name="ot2", tag="acc")
            mlast = 4 * j + 3
            for m in range(4 * j + 4):
                off = m * P - j * CH if m >= 4 * j else 0
                sc2 = bpsum.tile([P, 1024], F32, name="sc2", tag="sc")
                nc.tensor.matmul(
                    sc2[:, off:CH],
                    lhsT=k_pack[0:64, m * P : (m + 1) * P],
                    rhs=q_pack[0:64, j * CH + off : (j + 1) * CH],
                )
                nc.tensor.matmul(
                    sc2[:, CH + off : 1024],
                    lhsT=k_pack[64:128, m * P : (m + 1) * P],
                    rhs=q_pack[64:128, j * CH + off : (j + 1) * CH],
                )
                at2 = bwork.tile([P, 1024], BF16, name="at2")
                if off > 0:
                    sc_v = sc2.rearrange("p (h q) -> p h q", h=2)
                    at_v = at2.rearrange("p (h q) -> p h q", h=2)
                    nc.scalar.activation(
                        at_v[:, :, off:CH], sc_v[:, :, off:CH], EXP, scale=0.125
                    )
                else:
                    nc.scalar.activation(at2[:], sc2[:], EXP, scale=0.125)
                if m >= 4 * j:
                    nc.vector.tensor_mul(
                        at2[:, off : off + P], at2[:, off : off + P], trimask_sb[:]
                    )
                    nc.vector.tensor_mul(
                        at2[:, CH + off : CH + off + P],
                        at2[:, CH + off : CH + off + P],
                        trimask_sb[:],
                    )
                nc.tensor.matmul(
                    ot2[0:65, off:CH],
                    lhsT=vone_v[:, m, 2 * p, :],
                    rhs=at2[:, off:CH],
                    start=(m == 0),
                    stop=(m == mlast),
                )
                nc.tensor.matmul(
                    ot2[0:65, CH + off : 1024],
                    lhsT=vone_v[:, m, 2 * p + 1, :],
                    rhs=at2[:, CH + off : 1024],
                    start=(m == 0),
                    stop=(m == mlast),
                )
            # ---- evict OT halves + denominators (per J) ----
            nc.vector.tensor_copy(
                otn2[0:64, p * S + j * CH : p * S + (j + 1) * CH], ot2[0:64, 0:CH]
            )
            stgB = fxwork.tile([64, CH], BF16, name="stgB")
            nc.vector.tensor_copy(stgB[:], ot2[0:64, CH:1024])
            nc.scalar.dma_start(
                otn2[64:128, p * S + j * CH : p * S + (j + 1) * CH], stgB[:]
            )
            dslot = (p * NCH + j) * 1024
            nc.vector.tensor_copy(
                dall[0:1, dslot : dslot + 1024], ot2[64:65, 0:1024]
            )
            # reciprocal in partition-parallel layout
            dPj = fxwork.tile([P, 8], F32, name="dPj")
            nc.scalar.dma_start(
                dPj[:],
                dall[0:1, dslot : dslot + 1024].rearrange("o (a b) -> o a b", a=P),
            )
            dPq = fxwork.tile([P, 8], F32, name="dPq")
            nc.vector.reciprocal(dPq[:], dPj[:])
            dPc = fxwork.tile([P, 8], BF16, name="dPc")
            with nc.allow_low_precision("softmax denominators"):
                nc.vector.tensor_copy(dPc[:], dPq[:])
            nc.scalar.dma_start(
                dallinv[0:1, dslot : dslot + 1024].rearrange("o (a b) -> o a b", a=P),
                dPc[:],
            )

        def emit_final(p, j):
            dslot = (p * NCH + j) * 1024
            dvb = bpsum.tile([P, CH], F32, name="dvb", tag="proj")
            nc.tensor.matmul(
                dvb[:],
                lhsT=halfsel[0:1, 0:128],
                rhs=dallinv[0:1, dslot : dslot + CH],
                start=True,
                stop=False,
            )
            nc.tensor.matmul(
                dvb[:],
                lhsT=halfsel[0:1, 128:256],
                rhs=dallinv[0:1, dslot + CH : dslot + 1024],
                start=False,
                stop=True,
            )
            nc.vector.tensor_mul(
                otn2[:, p * S + j * CH : p * S + (j + 1) * CH],
                otn2[:, p * S + j * CH : p * S + (j + 1) * CH],
                dvb[:],
            )

        def emit_C(g):
            # out projection for q-tiles 4g..4g+3
            for qt_i in range(4 * g, 4 * g + 4):
                outsb = cwork.tile([P, D], F32, name="outsb")
                for dc in range(2):
                    pr = bpsum.tile([P, CH], F32, name="pr", tag="proj")
                    for p in range(NPAIR):
                        nc.tensor.matmul(
                            pr[:],
                            lhsT=otn2[:, p * S + qt_i * P : p * S + (qt_i + 1) * P],
                            rhs=wo_sb[:, p * D + dc * CH : p * D + (dc + 1) * CH],
                            start=(p == 0),
                            stop=(p == NPAIR - 1),
                        )
                    if dc == 0:
                        nc.vector.tensor_copy(outsb[:, 0:CH], pr[:])
                    else:
                        nc.scalar.copy(outsb[:, CH:D], pr[:])
                nc.sync.dma_start(out_d[qt_i * P : (qt_i + 1) * P, :], outsb[:])

        # ---------------- schedule ----------------
        # A(0) split around B(0,0)'s needs; A(j+1) pieces interleave inside
        # the j-loop so PE fills ACT-bound B segments; finals pipeline one
        # (p, j) behind; out-projection per chunk after pair-1 finals.
        emit_qkT(0, 0, wq_sb, 0, qt)
        emit_qkT(0, 0, wk_sb, 2, kt_sb)
        for st in range(4):
            emit_V(st)
        emit_qkT(0, 1, wq_sb, 1, qt)
        emit_qkT(0, 1, wk_sb, 3, kt_sb)
        pending = None

        def after_B(p, j):
            nonlocal pending
            if pending is not None:
                emit_final(*pending)
                if pending[0] == 1:
                    emit_C(pending[1])
            pending = (p, j)

        for j in range(NCH):
            emit_BJ(0, j)
            after_B(0, j)
            if j + 1 < NCH:
                emit_qkT(j + 1, 0, wq_sb, 0, qt)
                emit_qkT(j + 1, 0, wk_sb, 2, kt_sb)
                for st in range(4 * j + 4, 4 * j + 8):
                    emit_V(st)
            emit_BJ(1, j)
            after_B(1, j)
            if j + 1 < NCH:
                emit_qkT(j + 1, 1, wq_sb, 1, qt)
                emit_qkT(j + 1, 1, wk_sb, 3, kt_sb)
        emit_final(*pending)
        emit_C(pending[1])

    nc.compile()
    return nc


def get_program():
    if "v2" not in _PROGRAM_CACHE:
        _PROGRAM_CACHE["v2"] = build_program()
    return _PROGRAM_CACHE["v2"]


def _bf16(a):
    import ml_dtypes

    return np.ascontiguousarray(a).astype(ml_dtypes.bfloat16)


def _rope_perm64():
    """Partition layout r -> original hd dim. Pairs are (r, r+16) within
    each 32-block: block b holds pairs 16b..16b+15; r%32<16 -> real (even
    dim), else imag (odd dim)."""
    perm = np.empty(HD, dtype=np.int64)
    for r in range(HD):
        blk, j = divmod(r, 32)
        pair = blk * 16 + (j % 16)
        imag = j // 16
        perm[r] = 2 * pair + imag
    return perm


_PERM64 = _rope_perm64()


def prep_core_inputs(x, w_qkv, b_qkv, w_out, core, xt_cache):
    b = core // 4
    heads = [(core % 4) * HPC + i for i in range(HPC)]

    if b not in xt_cache:
        xb = np.asarray(x[b])  # [S, D]
        xt = np.ascontiguousarray(
            xb.T.reshape(KT, P, S).transpose(1, 0, 2).reshape(P, KT * S)
        )
        xt_cache[b] = _bf16(xt)
    xt = xt_cache[b]

    # permuted row indices for Q/K packs: pack pk rows = heads 2pk, 2pk+1
    def pack_rows(section, pk):
        rows = []
        for half in range(2):
            h = heads[2 * pk + half]
            rows.extend(section * D + h * HD + _PERM64)
        return rows

    def wT_tiles(rows):
        w_sel = w_qkv[rows]  # [128, 1024]
        # lhsT[p, k*128 + c] = w_sel[c, k*128+p]
        return np.ascontiguousarray(
            w_sel.T.reshape(KT, P, P).transpose(1, 0, 2).reshape(P, KT * P)
        )

    wq = np.concatenate([wT_tiles(pack_rows(0, pk)) for pk in range(NPAIR)], axis=1)
    wk = np.concatenate([wT_tiles(pack_rows(1, pk)) for pk in range(NPAIR)], axis=1)

    # V natural: rhs tiles [128 (D-chunk), 256 (4 heads x 64 natural)]
    vrows = []
    for h in heads:
        vrows.extend(range(2 * D + h * HD, 2 * D + (h + 1) * HD))
    wv_sel = w_qkv[vrows]  # [256, 1024]
    wv = np.ascontiguousarray(
        wv_sel.T.reshape(KT, P, 256).transpose(1, 0, 2).reshape(P, KT * 256)
    )

    # bias columns [128, 4]: (Q pk0, Q pk1, K pk0, K pk1)
    bqk = np.empty((P, 4), dtype=np.float32)
    for qk in range(2):
        for pk in range(NPAIR):
            rows = pack_rows(qk, pk)
            bqk[:, qk * 2 + pk] = b_qkv[rows]

    # rope tables in permuted-partition layout [128, S]
    dims = np.arange(0, HD, 2, dtype=np.float64)
    invf = 1.0 / (THETA ** (dims / HD))  # [32] per pair index
    pos = np.arange(S, dtype=np.float64)
    r = np.arange(HD)
    blk, j = r // 32, r % 32
    pair = blk * 16 + (j % 16)
    is_imag = (j % 32) >= 16
    freq = invf[pair]  # [64]
    ang = pos[None, :] * freq[:, None]  # [64, S]
    c64 = np.cos(ang)
    # S[r] = -sin for real, +sin for imag; S''[r] = S[partner(r)] = flipped
    s64 = np.sin(ang) * np.where(is_imag, -1.0, 1.0)[:, None]
    ropec = np.tile(c64, (2, 1))  # [128, S]
    ropes = np.tile(s64, (2, 1))

    trimask = np.triu(np.ones((P, P), dtype=np.float32))

    # wo[kk, p2*D + n] = w_out[n, gh*64 + kk%64], gh = heads[2*p2 + kk//64]
    wo = np.empty((P, NPAIR * D), dtype=np.float32)
    for p2 in range(NPAIR):
        for half in range(2):
            gh = heads[2 * p2 + half]
            wo[half * 64 : (half + 1) * 64, p2 * D : (p2 + 1) * D] = w_out[
                :, gh * HD : (gh + 1) * HD
            ].T
    return {
        "xt": xt,
        "wq": _bf16(wq),
        "wk": _bf16(wk),
        "wv": _bf16(wv),
        "bqk": np.ascontiguousarray(bqk),
        "ropec": _bf16(ropec),
        "ropes": _bf16(ropes),
        "trimask": _bf16(trimask),
        "wo": _bf16(wo),
    }


def kernel(x, w_qkv, b_qkv, w_out, b_out):
    global LAST_RESULTS
    x = np.asarray(x, dtype=np.float32)
    w_qkv = np.asarray(w_qkv, dtype=np.float32)
    b_qkv = np.asarray(b_qkv, dtype=np.float32)
    w_out = np.asarray(w_out, dtype=np.float32)
    b_out = np.asarray(b_out, dtype=np.float32)

    nc = get_program()
    xt_cache = {}
    in_maps = [
        prep_core_inputs(x, w_qkv, b_qkv, w_out, core, xt_cache)
        for core in range(NCORES)
    ]
    res = bass_utils.run_bass_kernel_spmd(
        nc, in_maps, core_ids=list(range(NCORES)), trace=TRACE
    )
    LAST_RESULTS = res
    partials = [r["outp"] for r in res.results]
    # v-bias contribution is constant across s (sum_k attn = 1):
    bconst = b_out + b_qkv[2 * D : 3 * D] @ w_out.T
    out = np.stack(
        [
            partials[0] + partials[1] + partials[2] + partials[3],
            partials[4] + partials[5] + partials[6] + partials[7],
        ]
    )
    out = out + bconst[None, None, :]
    return out.astype(np.float32)


# revision 9
# speedup vs baseline: 1.4442x; 1.0020x over previous
# Multi-head attention (RoPE, causal) Trainium2 Bass kernel, v2.
# B=2, S=2048, D=1024, 16 heads, hd=64, fp32 I/O.
#
# Sharding: 32 (batch, head) units over 8 cores -> each core gets one batch
# and 4 heads. Each core computes its 4 heads' attention output and the
# partial out-projection (sum over its heads); the host sums the 4 partials
# per batch and adds the bias constant.
#
# v2 vs v1: Q^T/K^T are produced directly transposed by the QKV projection
# (W stationary, X^T streaming) so no PE transposes are needed; RoPE runs in
# the transposed [hd, s] layout using a host-side W-row permutation that
# makes the rotation partner swap a single DVE stream_shuffle (swap the
# 16-halves of each 32-partition block); score matmuls are K=64 row-packed
# pairs (partitions 0:64 / 64:128) that execute concurrently on the PE;
# diagonal score blocks are N-trimmed; A/B phases are interleaved per
# 512-chunk to keep the PE dense.
#
# Self-contained: all shapes/sharding hardcoded; no sibling imports.

import numpy as np

import concourse.bass as bass  # noqa: F401
import concourse.mybir as mybir
import concourse.tile as tile
from concourse import bacc, bass_utils

F32 = mybir.dt.float32
BF16 = mybir.dt.bfloat16
EXP = mybir.ActivationFunctionType.Exp
ADD = mybir.AluOpType.add
MULT = mybir.AluOpType.mult

B = 2
S = 2048
D = 1024
NHEADS = 16
HD = 64
HPC = 4  # heads per core
NCORES = 8
NPAIR = 2  # head pairs per core
P = 128
CH = 512  # q chunk
NCH = S // CH  # 4
NT = S // P  # 16
KT = D // P  # 8
THETA = 10000.0

# swap the 16-halves of each 32-partition block (RoPE partner swap)
SWAP_MASK = list(range(16, 32)) + list(range(0, 16))

# module-level knobs for test harness
TRACE = False
LAST_RESULTS = None

_PROGRAM_CACHE = {}


def build_program():
    nc = bacc.Bacc(
        "TRN2", target_bir_lowering=False, debug=False, enable_asserts=False
    )

    # ---- DRAM I/O ----
    xt_d = nc.dram_tensor("xt", [P, KT * S], BF16, kind="ExternalInput").ap()
    wq_d = nc.dram_tensor("wq", [P, NPAIR * KT * P], BF16, kind="ExternalInput").ap()
    wk_d = nc.dram_tensor("wk", [P, NPAIR * KT * P], BF16, kind="ExternalInput").ap()
    wv_d = nc.dram_tensor("wv", [P, KT * 256], BF16, kind="ExternalInput").ap()
    bqk_d = nc.dram_tensor("bqk", [P, 4], F32, kind="ExternalInput").ap()
    ropec_d = nc.dram_tensor("ropec", [P, S], BF16, kind="ExternalInput").ap()
    ropes_d = nc.dram_tensor("ropes", [P, S], BF16, kind="ExternalInput").ap()
    trimask_d = nc.dram_tensor("trimask", [P, P], BF16, kind="ExternalInput").ap()
    wo_d = nc.dram_tensor("wo", [P, NPAIR * D], BF16, kind="ExternalInput").ap()
    out_d = nc.dram_tensor("outp", [S, D], F32, kind="ExternalOutput").ap()

    from contextlib import ExitStack

    with tile.TileContext(nc) as tc, ExitStack() as ctx:
        const = ctx.enter_context(tc.tile_pool(name="const", bufs=1))

        # weights for Q^T/K^T: lhsT tiles [128 (D-chunk k), 128 (pack rows)]
        wq_sb = const.tile([P, NPAIR * KT * P], BF16)
        wk_sb = const.tile([P, NPAIR * KT * P], BF16)
        wv_sb = const.tile([P, KT * 256], BF16)
        bqk_sb = const.tile([P, 4], F32)
        ropec_sb = const.tile([P, S], BF16)
        ropes_sb = const.tile([P, S], BF16)
        trimask_sb = const.tile([P, P], BF16)
        wo_sb = const.tile([P, NPAIR * D], BF16)
        # x^T, entire input: [128 (D-chunk k), k*S + t]
        xt_sb = const.tile([P, KT * S], BF16)

        # const loads: split across the two HWDGE rings; first-needed first.
        # sync ring: wq pack0, x chunk 0 k-slices, wq pack1, rest of x
        nc.sync.dma_start(wq_sb[:, 0 : KT * P], wq_d[:, 0 : KT * P])
        for k in range(KT):
            sl = slice(k * S, k * S + CH)
            nc.sync.dma_start(xt_sb[:, sl], xt_d[:, sl])
        nc.sync.dma_start(wq_sb[:, KT * P :], wq_d[:, KT * P :])
        for c in range(1, NCH):
            for k in range(KT):
                sl = slice(k * S + c * CH, k * S + (c + 1) * CH)
                nc.sync.dma_start(xt_sb[:, sl], xt_d[:, sl])
        # scalar/ACT ring: everything else
        nc.scalar.dma_start(wk_sb[:], wk_d[:])
        nc.scalar.dma_start(bqk_sb[:], bqk_d[:])
        nc.scalar.dma_start(ropec_sb[:], ropec_d[:])
        nc.scalar.dma_start(ropes_sb[:], ropes_d[:])
        nc.scalar.dma_start(wv_sb[:], wv_d[:])
        nc.scalar.dma_start(trimask_sb[:], trimask_d[:])
        nc.scalar.dma_start(wo_sb[:], wo_d[:])

        # persistent activations
        # Q^T / K^T packs: [128 (2 heads x 64 permuted hd), s] per pack
        qt = const.tile([P, NPAIR * S], BF16)
        kt_sb = const.tile([P, NPAIR * S], BF16)
        # vone: per s-tile [128, 4*65]; per head 64 V cols + ones col
        vone = const.tile([P, NT * (HPC * 65)], BF16)
        # O^T packs (normalized in place): [128 (2 heads*hd), s] per pair
        otn2 = const.tile([P, NPAIR * S], BF16)

        vone_v = vone.rearrange("p (t h c) -> p t h c", t=NT, h=HPC)
        nc.gpsimd.memset(vone_v[:, :, :, 64], 1.0)

        # half-selector columns for the rank-1 denominator broadcast
        halfsel = const.tile([1, 256], BF16)
        nc.gpsimd.memset(halfsel[:, 0:64], 1.0)
        nc.gpsimd.memset(halfsel[:, 64:192], 0.0)
        nc.gpsimd.memset(halfsel[:, 192:256], 1.0)

        # softmax denominator collection: per (pair, J): [headA 512 | headB 512]
        dall = const.tile([1, NPAIR * NCH * 1024], F32)
        dallinv = const.tile([1, NPAIR * NCH * 1024], BF16)

        bpsum = ctx.enter_context(tc.tile_pool(name="bpsum", bufs=2, space="PSUM"))
        opsum = ctx.enter_context(tc.tile_pool(name="opsum", bufs=1, space="PSUM"))
        awork = ctx.enter_context(tc.tile_pool(name="awork", bufs=3))
        bwork = ctx.enter_context(tc.tile_pool(name="bwork", bufs=4))
        fxwork = ctx.enter_context(tc.tile_pool(name="fxwork", bufs=2))
        cwork = ctx.enter_context(tc.tile_pool(name="cwork", bufs=3))

        # ---------------- Phase A ----------------
        def emit_qkT(c, pk, w_sb, bias_col, dst):
            """One 512-col chunk of a Q^T or K^T pack, with bias + RoPE."""
            ps = bpsum.tile([P, CH], F32, name="psq", tag="proj")
            for k in range(KT):
                nc.tensor.matmul(
                    ps[:],
                    lhsT=w_sb[:, (pk * KT + k) * P : (pk * KT + k + 1) * P],
                    rhs=xt_sb[:, k * S + c * CH : k * S + (c + 1) * CH],
                    start=(k == 0),
                    stop=(k == KT - 1),
                )
            # stage = ps + bias (per-partition)
            stage = awork.tile([P, CH], BF16, name="stage")
            nc.vector.tensor_scalar_add(stage[:], ps[:], bqk_sb[:, bias_col : bias_col + 1])
            # t1 = stage * S'' ; sw = shuffle(t1) ; rot = stage*C + sw
            t1 = awork.tile([P, CH], BF16, name="t1")
            nc.vector.tensor_mul(t1[:], stage[:], ropes_sb[:, c * CH : (c + 1) * CH])
            sw = awork.tile([P, CH], BF16, name="sw")
            nc.vector.stream_shuffle(sw[:], t1[:], SWAP_MASK)
            rot1 = awork.tile([P, CH], BF16, name="rot1")
            nc.vector.tensor_mul(rot1[:], stage[:], ropec_sb[:, c * CH : (c + 1) * CH])
            nc.vector.tensor_add(
                dst[:, pk * S + c * CH : pk * S + (c + 1) * CH], rot1[:], sw[:]
            )

        def emit_V(st):
            psv = bpsum.tile([P, 256], F32, name="psv", tag="proj")
            for k in range(KT):
                nc.tensor.matmul(
                    psv[:],
                    lhsT=xt_sb[:, k * S + st * P : k * S + (st + 1) * P],
                    rhs=wv_sb[:, k * 256 : (k + 1) * 256],
                    start=(k == 0),
                    stop=(k == KT - 1),
                )
            nc.scalar.copy(
                vone_v[:, st, :, 0:64],
                psv.rearrange("p (h c) -> p h c", h=HPC),
            )

        # ---------------- Phase B: attention ----------------
        def emit_BJ(p, j):
            q_pack = qt[:, p * S : (p + 1) * S]
            k_pack = kt_sb[:, p * S : (p + 1) * S]
            ot2 = opsum.tile([P, 1024], F32, name="ot2", tag="acc")
            mlast = 4 * j + 3
            for m in range(4 * j + 4):
                off = m * P - j * CH if m >= 4 * j else 0
                sc2 = bpsum.tile([P, 1024], F32, name="sc2", tag="sc")
                nc.tensor.matmul(
                    sc2[:, off:CH],
                    lhsT=k_pack[0:64, m * P : (m + 1) * P],
                    rhs=q_pack[0:64, j * CH + off : (j + 1) * CH],
                )
                nc.tensor.matmul(
                    sc2[:, CH + off : 1024],
                    lhsT=k_pack[64:128, m * P : (m + 1) * P],
                    rhs=q_pack[64:128, j * CH + off : (j + 1) * CH],
                )
                at2 = bwork.tile([P, 1024], BF16, name="at2")
                if off > 0:
                    sc_v = sc2.rearrange("p (h q) -> p h q", h=2)
                    at_v = at2.rearrange("p (h q) -> p h q", h=2)
                    nc.scalar.activation(
                        at_v[:, :, off:CH], sc_v[:, :, off:CH], EXP, scale=0.125
                    )
                else:
                    nc.scalar.activation(at2[:], sc2[:], EXP, scale=0.125)
                if m >= 4 * j:
                    nc.vector.tensor_mul(
                        at2[:, off : off + P], at2[:, off : off + P], trimask_sb[:]
                    )
                    nc.vector.tensor_mul(
                        at2[:, CH + off : CH + off + P],
                        at2[:, CH + off : CH + off + P],
                        trimask_sb[:],
                    )
                nc.tensor.matmul(
                    ot2[0:65, off:CH],
                    lhsT=vone_v[:, m, 2 * p, :],
                    rhs=at2[:, off:CH],
                    start=(m == 0),
                    stop=(m == mlast),
                )
                nc.tensor.matmul(
                    ot2[0:65, CH + off : 1024],
                    lhsT=vone_v[:, m, 2 * p + 1, :],
                    rhs=at2[:, CH + off : 1024],
                    start=(m == 0),
                    stop=(m == mlast),
                )
            # ---- evict OT halves + denominators (per J) ----
            nc.scalar.copy(
                otn2[0:64, p * S + j * CH : p * S + (j + 1) * CH], ot2[0:64, 0:CH]
            )
            stgB = fxwork.tile([64, CH], BF16, name="stgB")
            nc.vector.tensor_copy(stgB[:], ot2[0:64, CH:1024])
            nc.sync.dma_start(
                otn2[64:128, p * S + j * CH : p * S + (j + 1) * CH], stgB[:]
            )
            dslot = (p * NCH + j) * 1024
            nc.vector.tensor_copy(
                dall[0:1, dslot : dslot + 1024], ot2[64:65, 0:1024]
            )
            # reciprocal in partition-parallel layout
            dPj = fxwork.tile([P, 8], F32, name="dPj")
            nc.sync.dma_start(
                dPj[:],
                dall[0:1, dslot : dslot + 1024].rearrange("o (a b) -> o a b", a=P),
            )
            dPq = fxwork.tile([P, 8], F32, name="dPq")
            nc.vector.reciprocal(dPq[:], dPj[:])
            dPc = fxwork.tile([P, 8], BF16, name="dPc")
            with nc.allow_low_precision("softmax denominators"):
                nc.vector.tensor_copy(dPc[:], dPq[:])
            nc.sync.dma_start(
                dallinv[0:1, dslot : dslot + 1024].rearrange("o (a b) -> o a b", a=P),
                dPc[:],
            )

        def emit_final(p, j):
            dslot = (p * NCH + j) * 1024
            dvb = bpsum.tile([P, CH], F32, name="dvb", tag="proj")
            nc.tensor.matmul(
                dvb[:],
                lhsT=halfsel[0:1, 0:128],
                rhs=dallinv[0:1, dslot : dslot + CH],
                start=True,
                stop=False,
            )
            nc.tensor.matmul(
                dvb[:],
                lhsT=halfsel[0:1, 128:256],
                rhs=dallinv[0:1, dslot + CH : dslot + 1024],
                start=False,
                stop=True,
            )
            nc.vector.tensor_mul(
                otn2[:, p * S + j * CH : p * S + (j + 1) * CH],
                otn2[:, p * S + j * CH : p * S + (j + 1) * CH],
                dvb[:],
            )

        def emit_C(g):
            # out projection for q-tiles 4g..4g+3
            for qt_i in range(4 * g, 4 * g + 4):
                outsb = cwork.tile([P, D], F32, name="outsb")
                for dc in range(2):
                    pr = bpsum.tile([P, CH], F32, name="pr", tag="proj")
                    for p in range(NPAIR):
                        nc.tensor.matmul(
                            pr[:],
                            lhsT=otn2[:, p * S + qt_i * P : p * S + (qt_i + 1) * P],
                            rhs=wo_sb[:, p * D + dc * CH : p * D + (dc + 1) * CH],
                            start=(p == 0),
                            stop=(p == NPAIR - 1),
                        )
                    if dc == 0:
                        nc.vector.tensor_copy(outsb[:, 0:CH], pr[:])
                    else:
                        nc.scalar.copy(outsb[:, CH:D], pr[:])
                nc.gpsimd.dma_start(out_d[qt_i * P : (qt_i + 1) * P, :], outsb[:])

        # ---------------- schedule ----------------
        # A(0) split around B(0,0)'s needs; A(j+1) pieces interleave inside
        # the j-loop so PE fills ACT-bound B segments; finals pipeline one
        # (p, j) behind; out-projection per chunk after pair-1 finals.
        emit_qkT(0, 0, wq_sb, 0, qt)
        emit_qkT(0, 0, wk_sb, 2, kt_sb)
        for st in range(4):
            emit_V(st)
        emit_qkT(0, 1, wq_sb, 1, qt)
        emit_qkT(0, 1, wk_sb, 3, kt_sb)
        pending = None

        def after_B(p, j):
            nonlocal pending
            if pending is not None:
                emit_final(*pending)
                if pending[0] == 1:
                    emit_C(pending[1])
            pending = (p, j)

        for j in range(NCH):
            emit_BJ(0, j)
            after_B(0, j)
            if j + 1 < NCH:
                emit_qkT(j + 1, 0, wq_sb, 0, qt)
                emit_qkT(j + 1, 0, wk_sb, 2, kt_sb)
                for st in range(4 * j + 4, 4 * j + 8):
                    emit_V(st)
            emit_BJ(1, j)
            after_B(1, j)
            if j + 1 < NCH:
                emit_qkT(j + 1, 1, wq_sb, 1, qt)
                emit_qkT(j + 1, 1, wk_sb, 3, kt_sb)
        emit_final(*pending)
        emit_C(pending[1])

    nc.compile()
    return nc


def get_program():
    if "v2" not in _PROGRAM_CACHE:
        _PROGRAM_CACHE["v2"] = build_program()
    return _PROGRAM_CACHE["v2"]


def _bf16(a):
    import ml_dtypes

    return np.ascontiguousarray(a).astype(ml_dtypes.bfloat16)


def _rope_perm64():
    """Partition layout r -> original hd dim. Pairs are (r, r+16) within
    each 32-block: block b holds pairs 16b..16b+15; r%32<16 -> real (even
    dim), else imag (odd dim)."""
    perm = np.empty(HD, dtype=np.int64)
    for r in range(HD):
        blk, j = divmod(r, 32)
        pair = blk * 16 + (j % 16)
        imag = j // 16
        perm[r] = 2 * pair + imag
    return perm


_PERM64 = _rope_perm64()


def prep_core_inputs(x, w_qkv, b_qkv, w_out, core, xt_cache):
    b = core // 4
    heads = [(core % 4) * HPC + i for i in range(HPC)]

    if b not in xt_cache:
        xb = np.asarray(x[b])  # [S, D]
        xt = np.ascontiguousarray(
            xb.T.reshape(KT, P, S).transpose(1, 0, 2).reshape(P, KT * S)
        )
        xt_cache[b] = _bf16(xt)
    xt = xt_cache[b]

    # permuted row indices for Q/K packs: pack pk rows = heads 2pk, 2pk+1
    def pack_rows(section, pk):
        rows = []
        for half in range(2):
            h = heads[2 * pk + half]
            rows.extend(section * D + h * HD + _PERM64)
        return rows

    def wT_tiles(rows):
        w_sel = w_qkv[rows]  # [128, 1024]
        # lhsT[p, k*128 + c] = w_sel[c, k*128+p]
        return np.ascontiguousarray(
            w_sel.T.reshape(KT, P, P).transpose(1, 0, 2).reshape(P, KT * P)
        )

    wq = np.concatenate([wT_tiles(pack_rows(0, pk)) for pk in range(NPAIR)], axis=1)
    wk = np.concatenate([wT_tiles(pack_rows(1, pk)) for pk in range(NPAIR)], axis=1)

    # V natural: rhs tiles [128 (D-chunk), 256 (4 heads x 64 natural)]
    vrows = []
    for h in heads:
        vrows.extend(range(2 * D + h * HD, 2 * D + (h + 1) * HD))
    wv_sel = w_qkv[vrows]  # [256, 1024]
    wv = np.ascontiguousarray(
        wv_sel.T.reshape(KT, P, 256).transpose(1, 0, 2).reshape(P, KT * 256)
    )

    # bias columns [128, 4]: (Q pk0, Q pk1, K pk0, K pk1)
    bqk = np.empty((P, 4), dtype=np.float32)
    for qk in range(2):
        for pk in range(NPAIR):
            rows = pack_rows(qk, pk)
            bqk[:, qk * 2 + pk] = b_qkv[rows]

    # rope tables in permuted-partition layout [128, S]
    dims = np.arange(0, HD, 2, dtype=np.float64)
    invf = 1.0 / (THETA ** (dims / HD))  # [32] per pair index
    pos = np.arange(S, dtype=np.float64)
    r = np.arange(HD)
    blk, j = r // 32, r % 32
    pair = blk * 16 + (j % 16)
    is_imag = (j % 32) >= 16
    freq = invf[pair]  # [64]
    ang = pos[None, :] * freq[:, None]  # [64, S]
    c64 = np.cos(ang)
    # S[r] = -sin for real, +sin for imag; S''[r] = S[partner(r)] = flipped
    s64 = np.sin(ang) * np.where(is_imag, -1.0, 1.0)[:, None]
    ropec = np.tile(c64, (2, 1))  # [128, S]
    ropes = np.tile(s64, (2, 1))

    trimask = np.triu(np.ones((P, P), dtype=np.float32))

    # wo[kk, p2*D + n] = w_out[n, gh*64 + kk%64], gh = heads[2*p2 + kk//64]
    wo = np.empty((P, NPAIR * D), dtype=np.float32)
    for p2 in range(NPAIR):
        for half in range(2):
            gh = heads[2 * p2 + half]
            wo[half * 64 : (half + 1) * 64, p2 * D : (p2 + 1) * D] = w_out[
                :, gh * HD : (gh + 1) * HD
            ].T
    return {
        "xt": xt,
        "wq": _bf16(wq),
        "wk": _bf16(wk),
        "wv": _bf16(wv),
        "bqk": np.ascontiguousarray(bqk),
        "ropec": _bf16(ropec),
        "ropes": _bf16(ropes),
        "trimask": _bf16(trimask),
        "wo": _bf16(wo),
    }


def kernel(x, w_qkv, b_qkv, w_out, b_out):
    global LAST_RESULTS
    x = np.asarray(x, dtype=np.float32)
    w_qkv = np.asarray(w_qkv, dtype=np.float32)
    b_qkv = np.asarray(b_qkv, dtype=np.float32)
    w_out = np.asarray(w_out, dtype=np.float32)
    b_out = np.asarray(b_out, dtype=np.float32)

    nc = get_program()
    xt_cache = {}
    in_maps = [
        prep_core_inputs(x, w_qkv, b_qkv, w_out, core, xt_cache)
        for core in range(NCORES)
    ]
    res = bass_utils.run_bass_kernel_spmd(
        nc, in_maps, core_ids=list(range(NCORES)), trace=TRACE
    )
    LAST_RESULTS = res
    partials = [r["outp"] for r in res.results]
    # v-bias contribution is constant across s (sum_k attn = 1):
    bconst = b_out + b_qkv[2 * D : 3 * D] @ w_out.T
    out = np.stack(
        [
            partials[0] + partials[1] + partials[2] + partials[3],
            partials[4] + partials[5] + partials[6] + partials[7],
        ]
    )
    out = out + bconst[None, None, :]
    return out.astype(np.float32)
